# revision 1
# baseline (speedup 1.0000x reference)
"""GAttentionBlock (GroupNorm + 8-head self-attention + proj + residual) on 8
Trainium2 NeuronCores, data-parallel over batch (B=8 -> 1 image per core).

v2: fp8(e4m3)+DoubleRow rework of the bf16 baseline. DoubleRow halves the
per-K-split cost wherever the contraction has >=2 128-tiles, so:
  - qkv: xn stored fp8 [128, 6, T] (C padded 640->768; row 640 is a constant-1
    bias row, chunk 5 otherwise zero), weights fp8 x16; 3 DR matmuls per
    512-half instead of 5 bf16 ones. q/k psum copied to bf16 [128(d pad), H, T]
    (d padded to 128 with zero weight cols so scores can contract K=128).
  - scores: single K-split (d=80) so DR cannot help; plain bf16 matmuls with
    FWL. exp on ACT with scale 1/(256*sqrt(80)) and bias -2 (keeps exp in
    e4m3 range; denominator comes from the same quantized probs so the shift
    cancels), probs written fp8.
  - AV: vT fp8 [128, H, 8sc, 112] (x16, ones col at 96 emits the softmax
    denominator at psum partition 96), probs fp8: 4 DR matmuls over sc pairs
    instead of 8.
  - proj: a_sb fp8 [81, H, T] (x16; row 80 constant 1), pwT fp8 [81, H, C]
    (x16; row 80 of head 0 carries 16*proj_b): 4 DR matmuls over head pairs;
    out = psum/256 + x fused in one DVE scalar_tensor_tensor.
All biases are folded into matmul rows; ACT runs only exp (the new bottleneck
engine); psum->sbuf copies are split DVE/GpSimd. Scaling: w,v,a,pw carry x16
into fp8 so ~N(0,1/C) weights land in e4m3's normal range; the net 256x on
scores/proj is divided back in the exp scale / output scale.
"""
import copy

import numpy as np
import ml_dtypes

import concourse.bass as bass
import concourse.mybir as mybir
import concourse.tile as tile
from concourse.bass_utils import run_bass_kernel_spmd

F32 = mybir.dt.float32
BF16 = mybir.dt.bfloat16
F8 = mybir.dt.float8e4

B, C, HH, WW = 8, 640, 32, 32
T = HH * WW            # 1024
NH, D = 8, 80          # heads, head dim
DP = 128               # padded head dim (q/k)
G = 32                 # groupnorm groups
GS = C // G            # 20 channels per group
EPS = 1e-5
NCHUNK = C // 128      # 5 channel chunks of 128
CP = 768               # padded channels (bias row at 640)
NCHP = CP // 128       # 6
NSC = T // 128         # 8 sequence chunks of 128
VW = 112               # vT row width (80 v + pad + ones col at 96)
QCOLS = NH * DP        # 1024
VBASE = 2 * QCOLS      # 2048
NCOLS = 2 * QCOLS + NH * D  # 2688
WSCALE = 16.0
EXPSHIFT = -2.0
EXPSCALE = 1.0 / (WSCALE * WSCALE * np.sqrt(np.float64(D)))

_MAXW = 1
GP_XN = False  # gpsimd tensor_scalar measured ~15us on HW (10x the model) — keep Q7 out of the loop


def _split_multiwait(nc):
    """This walrus build rejects >1 sync-wait command per instruction. Move
    extra waits onto same-engine NoOps inserted just before the instruction."""
    ctr = 0
    new_module = copy.replace(nc.m, functions=[])
    for function in nc.m.functions:
        new_function = copy.replace(function, blocks=[])
        new_function.set_allocations_from_list(function.allocations)
        for block in function.blocks:
            new_insts = []
            for inst in block.instructions:
                si = inst.sync_info
                ow = list(si.on_wait) if (si is not None and si.on_wait) else []
                if len(ow) > _MAXW:
                    head, tail = ow[:-_MAXW], ow[-_MAXW:]
                    for w in head:
                        ctr += 1
                        new_insts.append(mybir.InstNoOp(
                            name=f"mwsplit_{ctr}",
                            engine=inst.engine,
                            sync_info=mybir.SyncInfo(on_wait=[w], on_update=[]),
                            bass_nofuse=True,
                        ))
                    inst.sync_info = mybir.SyncInfo(
                        on_wait=tail,
                        on_update=list(si.on_update) if si.on_update else [],
                    )
                new_insts.append(inst)
            new_function.blocks.append(copy.replace(block, instructions=new_insts))
        new_module.functions.append(new_function)
    nc.m = new_module


def _build_program(repeat=1, loop_n=0, split_mw=True, phase=5):
    nc = bass.Bass("TRN2", target_bir_lowering=False, num_devices=8)
    DR = mybir.MatmulPerfMode.DoubleRow

    x_d = nc.dram_tensor("x", [C, T], F32, kind="ExternalInput").ap()
    wqkv_d = nc.dram_tensor("wqkv8", [CP, NCOLS], F8, kind="ExternalInput").ap()
    pwT_d = nc.dram_tensor("pwT8", [97, NH, C], F8, kind="ExternalInput").ap()
    nw_d = nc.dram_tensor("nw", [C], F32, kind="ExternalInput").ap()
    nb_d = nc.dram_tensor("nb", [C], F32, kind="ExternalInput").ap()
    ind1_d = nc.dram_tensor("ind1", [C, G], F32, kind="ExternalInput").ap()
    ind2_d = nc.dram_tensor("ind2", [G, C], F32, kind="ExternalInput").ap()
    o_d = nc.dram_tensor("o", [C, T], F32, kind="ExternalOutput").ap()

    x_dv = x_d.rearrange("(o p) t -> p o t", p=128)       # [128, 5, 1024]
    o_dv = o_d.rearrange("(o p) t -> p o t", p=128)

    with tile.TileContext(nc) as tc:
        with tc.tile_pool(name="wpool", bufs=1) as wp, \
             tc.tile_pool(name="data", bufs=1) as dp, \
             tc.tile_pool(name="ptile", bufs=2) as pp, \
             tc.tile_pool(name="small", bufs=2) as sp, \
             tc.tile_pool(name="ps", bufs=2, space="PSUM") as ps, \
             tc.tile_pool(name="dram", bufs=2, space="DRAM") as dr:

            # ---------- weight / constant loads (ACT DMA queue: keeps the
            # SP queue free for the per-iteration x loads) ----------
            nwb = wp.tile([128, NCHUNK, 2], F32)
            nc.scalar.dma_start(out=nwb[:, :, 0], in_=nw_d.rearrange("(o p) -> p o", p=128))
            nc.scalar.dma_start(out=nwb[:, :, 1], in_=nb_d.rearrange("(o p) -> p o", p=128))

            ind1 = wp.tile([128, NCHUNK, G], F32)   # [channel -> group] one-hot
            ind2 = wp.tile([G, NCHUNK, 128], F32)   # [group -> channel] one-hot
            nc.scalar.dma_start(out=ind1, in_=ind1_d.rearrange("(o p) g -> p o g", p=128))
            nc.scalar.dma_start(out=ind2, in_=ind2_d.rearrange("g (o p) -> g o p", p=128))

            wqkv8 = wp.tile([128, NCHP, NCOLS], F8)
            nc.scalar.dma_start(out=wqkv8, in_=wqkv_d.rearrange("(o p) n -> p o n", p=128))
            pwT8 = wp.tile([97, NH, C], F8)
            nc.scalar.dma_start(out=pwT8, in_=pwT_d)

            eps_t = wp.tile([G, 1], F32)
            nc.vector.memset(eps_t, EPS)
            neg2 = wp.tile([128, 1], F32)
            nc.vector.memset(neg2, EXPSHIFT)
            ones80 = wp.tile([1, D], BF16)
            nc.vector.memset(ones80, 1.0)

            # ---------- persistent data tiles + one-time inits ----------
            # x/xn double-buffered across repeat-bodies so body k+1's input
            # load + GN overlap body k's attention phase
            nxb = min(repeat, 2)
            xn_bufs, x_bufs = [], []
            for bi in range(nxb):
                xnb = dp.tile([128, NCHP, T], F8, name=f"xn{bi}")
                nc.gpsimd.memset(xnb[:, NCHP - 1, :], 0.0)
                nc.gpsimd.memset(xnb[0:1, NCHP - 1, :], 1.0)  # bias row 640
                xn_bufs.append(xnb)
                x_bufs.append(dp.tile([128, NCHUNK, T], F32, name=f"x_sb{bi}"))
            vT = dp.tile([128, NH, NSC, VW], F8)
            nc.gpsimd.memset(vT[:, :, :, D:VW], 0.0)
            nc.gpsimd.memset(vT[:, :, :, 96:97], 1.0)     # denominator ones col
            a_sb = dp.tile([97, NH, T], F8)
            nc.gpsimd.memset(a_sb[64:97, :, :], 0.0)
            nc.gpsimd.memset(a_sb[96:97, :, :], 1.0)      # proj bias row
            q_sb = dp.tile([128, NH, T], BF16)
            k_sb = dp.tile([128, NH, T], BF16)

            import contextlib
            loop_cm = tc.For_i(0, loop_n, 1) if loop_n else contextlib.nullcontext()
            with loop_cm:
              for _rep in range(repeat):
                xn = xn_bufs[_rep % nxb]
                x_sb = x_bufs[_rep % nxb]
                # ---------- stage A: load x + GroupNorm ----------
                for j in range(NCHUNK):
                    nc.sync.dma_start(out=x_sb[:, j, :], in_=x_dv[:, j, :])

                stats = sp.tile([128, 2, 6], F32, tag="gn_stats")
                ss = sp.tile([128, NCHUNK, 2], F32, tag="gn_ss")
                for j in range(NCHUNK):
                    nc.vector.bn_stats(out=stats[:, 0, :], in_=x_sb[:, j, 0:512])
                    nc.vector.bn_stats(out=stats[:, 1, :], in_=x_sb[:, j, 512:1024])
                    nc.vector.bn_aggr(out=ss[:, j, :], in_=stats)
                    # ss[...,1] currently var; make it var + mean^2 = E[x^2]
                    nc.vector.scalar_tensor_tensor(
                        out=ss[:, j, 1:2], in0=ss[:, j, 0:1],
                        scalar=ss[:, j, 0:1], in1=ss[:, j, 1:2],
                        op0=mybir.AluOpType.mult, op1=mybir.AluOpType.add)

                ps_g = ps.tile([G, 2], F32, tag="work", bufs=2)
                for j in range(NCHUNK):
                    nc.tensor.matmul(ps_g, lhsT=ind1[:, j, :], rhs=ss[:, j, :],
                                     start=(j == 0), stop=(j == NCHUNK - 1))
                # group stats -> mean_g, rstd_g
                gm = sp.tile([G, 2], F32, tag="gn_gm")       # [mean_g, rstd_g]
                tmp_g = sp.tile([G, 2], F32, tag="gn_tmp")
                nc.vector.tensor_scalar_mul(gm, ps_g, 1.0 / GS)           # [mean, E2]
                nc.vector.tensor_tensor(out=tmp_g[:, 0:1], in0=gm[:, 0:1],
                                        in1=gm[:, 0:1], op=mybir.AluOpType.mult)
                nc.vector.tensor_tensor(out=tmp_g[:, 1:2], in0=gm[:, 1:2],
                                        in1=tmp_g[:, 0:1], op=mybir.AluOpType.subtract)
                nc.scalar.activation(out=tmp_g[:, 1:2], in_=tmp_g[:, 1:2],
                                     func=mybir.ActivationFunctionType.Ln,
                                     bias=eps_t, scale=1.0)
                nc.scalar.activation(out=gm[:, 1:2], in_=tmp_g[:, 1:2],
                                     func=mybir.ActivationFunctionType.Exp,
                                     scale=-0.5)   # rstd_g = (var+eps)^-0.5

                ab = sp.tile([128, NCHUNK, 2], F32, tag="gn_ab")
                for j in range(NCHUNK):
                    ps_bc = ps.tile([128, 2], F32, tag="work", bufs=2, name=f"ps_bc{j}")
                    nc.tensor.matmul(ps_bc, lhsT=ind2[:, j, :], rhs=gm,
                                     start=True, stop=True)
                    # A = rstd_c * norm_w ; B = norm_b - mean_c * A
                    nc.vector.tensor_tensor(out=ab[:, j, 0:1], in0=ps_bc[:, 1:2],
                                            in1=nwb[:, j, 0:1], op=mybir.AluOpType.mult)
                    nc.vector.scalar_tensor_tensor(
                        out=stats[:, 0, 0:1], in0=ps_bc[:, 0:1],
                        scalar=ab[:, j, 0:1], in1=nwb[:, j, 1:2],
                        op0=mybir.AluOpType.mult, op1=mybir.AluOpType.subtract)
                    nc.vector.tensor_scalar_mul(ab[:, j, 1:2],
                                                stats[:, 0, 0:1], -1.0)
                    if j % 2 == 0:
                        # ACT is idle during the GN head: apply xn there
                        nc.scalar.activation(
                            out=xn[:, j, :], in_=x_sb[:, j, :],
                            func=mybir.ActivationFunctionType.Identity,
                            bias=ab[:, j, 1:2], scale=ab[:, j, 0:1])
                    else:
                        nc.vector.tensor_scalar(
                            out=xn[:, j, :], in0=x_sb[:, j, :],
                            scalar1=ab[:, j, 0:1], scalar2=ab[:, j, 1:2],
                            op0=mybir.AluOpType.mult,
                            op1=mybir.AluOpType.add)

                # ---------- stages B+C: software-pipelined qkv + attention ----
                # PE is in-order and co-saturated with ACT, so each head
                # iteration interleaves stall-free "filler" matmul steps
                # (qk(h+1), AV(h-1), vT chains) between the scores pairs,
                # which are the only PE ops that wait on ACT (psum reuse).
                # Chains never span an iteration (pool round-robin + in-order
                # PE would deadlock); fillers are emitted chain-by-chain.
                p_tiles = {}

                def qk_steps(h):
                    steps = []
                    for w in range(2):  # 0=q, 1=k
                        dst = q_sb if w == 0 else k_sb
                        for tt in range(2):
                            cell = {}

                            def step(jp, w=w, tt=tt, dst=dst, cell=cell, h=h):
                                if jp == 0:
                                    cell['ps'] = ps.tile(
                                        [128, 512], F32, tag="work", bufs=2,
                                        name=f"ps_qk{h}_{w}_{tt}")
                                nc.tensor.matmul(
                                    cell['ps'],
                                    lhsT=wqkv8[:, 2 * jp:2 * jp + 2,
                                               w * QCOLS + h * DP:
                                               w * QCOLS + (h + 1) * DP],
                                    rhs=xn[:, 2 * jp:2 * jp + 2,
                                           tt * 512:(tt + 1) * 512],
                                    start=(jp == 0), stop=(jp == 2),
                                    perf_mode=DR)
                                if jp == 2:
                                    nc.vector.tensor_copy(
                                        out=dst[:, h, tt * 512:(tt + 1) * 512],
                                        in_=cell['ps'])
                            steps += [lambda jp=jp, s=step: s(jp)
                                      for jp in range(3)]
                    return steps

                def vT_steps(nn, scs):  # nn: 4-head group; scs: sc chunks
                    steps = []
                    for sc in scs:
                        cell = {}

                        def step(jp, sc=sc, nn=nn, cell=cell):
                            if jp == 0:
                                cell['ps'] = ps.tile(
                                    [128, 4 * D], F32, tag="work", bufs=2,
                                    name=f"ps_v{sc}_{nn}")
                            nc.tensor.matmul(
                                cell['ps'],
                                lhsT=xn[:, 2 * jp:2 * jp + 2,
                                        sc * 128:(sc + 1) * 128],
                                rhs=wqkv8[:, 2 * jp:2 * jp + 2,
                                          VBASE + nn * 4 * D:
                                          VBASE + (nn + 1) * 4 * D],
                                start=(jp == 0), stop=(jp == 2),
                                perf_mode=DR)
                            if jp == 2:
                                nc.vector.tensor_copy(
                                    out=vT[:, nn * 4:(nn + 1) * 4, sc, 0:D],
                                    in_=cell['ps'].rearrange(
                                        "p (h d) -> p h d", h=4))
                        steps += [lambda jp=jp, s=step: s(jp) for jp in range(3)]
                    return steps

                def av_steps(h):
                    # DRAM-bounce broadcast measured ~3us latency per head on
                    # HW and stalls the in-order DVE stream; broadcast via a
                    # tiny PE matmul + ACT psum evacuation instead. Trailers
                    # are separate filler steps so the ACT copy lands in the
                    # exp stream only after its wait is already satisfied.
                    p_t = p_tiles.pop(h)
                    rinv = sp.tile([1, T], BF16, tag="rinv", name=f"rinv{h}")
                    rb = sp.tile([D, T], BF16, tag="rb", name=f"rb{h}")
                    steps = []
                    cells = [{}, {}]
                    for tt in range(2):
                        sl = slice(tt * 512, (tt + 1) * 512)
                        cell = cells[tt]

                        def step(scp, sl=sl, cell=cell, h=h, p_t=p_t,
                                 rinv=rinv, tt=tt):
                            if scp == 0:
                                cell['ps'] = ps.tile(
                                    [VW, 512], F32, tag="av",
                                    name=f"ps_a{h}_{tt}", bufs=2)
                            ps_a = cell['ps']
                            nc.tensor.matmul(
                                ps_a,
                                lhsT=vT[:, h, 2 * scp:2 * scp + 2, :],
                                rhs=p_t[:, 2 * scp:2 * scp + 2, sl],
                                start=(scp == 0), stop=(scp == 3),
                                perf_mode=DR)
                            if scp == 3:
                                with nc.allow_low_precision(
                                        reason="softmax denom"):
                                    nc.vector.reciprocal(out=rinv[0:1, sl],
                                                         in_=ps_a[96:97, :])
                        steps += [lambda scp=scp, s=step: s(scp)
                                  for scp in range(4)]

                    r_dr = dr.tile([1, T], BF16, tag="rbounce", name=f"r_dr{h}")

                    def trailer(tt, h=h, rinv=rinv, rb=rb, r_dr=r_dr,
                                cells=cells):
                        sl = slice(tt * 512, (tt + 1) * 512)
                        if h == NH - 1:
                            # tail: PE-broadcast + ACT evacuation (ACT idle)
                            ps_rb = ps.tile([D, 512], F32, tag="work", bufs=2,
                                            name=f"ps_rb{h}_{tt}")
                            nc.tensor.matmul(ps_rb, lhsT=ones80,
                                             rhs=rinv[0:1, sl],
                                             start=True, stop=True)
                            nc.scalar.copy(out=rb[:, sl], in_=ps_rb)
                        else:
                            nc.sync.dma_start(out=r_dr[:, sl], in_=rinv[:, sl])
                            nc.sync.dma_start(
                                out=rb[:, sl],
                                in_=r_dr[0:1, sl].to_broadcast([D, 512]))
                        nc.vector.tensor_tensor(
                            out=a_sb[0:D, h, sl], in0=cells[tt]['ps'][0:D, :],
                            in1=rb[:, sl], op=mybir.AluOpType.mult)

                    steps.append(lambda: trailer(0))
                    steps.append(lambda: trailer(1))
                    return steps

                def run_iter(h, fillers):
                    p_t = pp.tile([128, NSC, T], F8, tag="probs", name=f"p_t{h}")
                    p_tiles[h] = p_t
                    nf = len(fillers)
                    done = 0
                    for sc in range(NSC):
                        want = (sc * nf) // NSC
                        while done < want:
                            fillers[done]()
                            done += 1
                        ps_s = ps.tile([128, T], F32, tag="scores",
                                       name=f"ps_s{h}_{sc}", bufs=2)
                        for tt in range(2):
                            nc.tensor.matmul(
                                ps_s[:, tt * 512:(tt + 1) * 512],
                                lhsT=k_sb[:, h, sc * 128:(sc + 1) * 128],
                                rhs=q_sb[:, h, tt * 512:(tt + 1) * 512],
                                start=True, stop=True)
                        nc.scalar.activation(out=p_t[:, sc, :], in_=ps_s,
                                             func=mybir.ActivationFunctionType.Exp,
                                             bias=neg2, scale=float(EXPSCALE))
                    while done < nf:
                        fillers[done]()
                        done += 1

                if phase == 2:
                    # timing-attribution subset: all qkv/vT chains, no attn
                    for h in range(NH):
                        for s in qk_steps(h):
                            s()
                    for s in vT_steps(0, range(NSC)) + vT_steps(1, range(NSC)):
                        s()
                elif phase == 3:
                    for s in qk_steps(0):
                        s()
                    run_iter(0, qk_steps(1) + vT_steps(0, range(NSC)))
                    nn1 = vT_steps(1, range(NSC))
                    for h in range(1, NH):
                        fillers = []
                        if h + 1 < NH:
                            fillers += qk_steps(h + 1)
                        if 1 <= h <= 4:
                            fillers += nn1[(h - 1) * 6:h * 6]
                        run_iter(h, fillers)
                        p_tiles.pop(h - 1, None)
                elif phase >= 4:
                    for s in qk_steps(0):
                        s()
                    run_iter(0, qk_steps(1) + vT_steps(0, range(NSC)))
                    nn1 = vT_steps(1, range(NSC))  # 24 steps: spread h=1..4
                    for h in range(1, NH):
                        fillers = []
                        if h + 1 < NH:
                            fillers += qk_steps(h + 1)
                        fillers += av_steps(h - 1)
                        if 1 <= h <= 4:
                            fillers += nn1[(h - 1) * 6:h * 6]
                        run_iter(h, fillers)
                    for s in av_steps(NH - 1):
                        s()

                # ---------- stage D: proj + bias + residual ----------
                for j in range(NCHUNK if phase >= 5 else 0):
                    out_t = sp.tile([128, T], F32, tag="out", name=f"out{j}")
                    for tt in range(2):
                        ptag = ("work", "scores", "av")[(2 * j + tt) % 3]
                        ps_p = ps.tile([128, 512], F32, tag=ptag,
                                       bufs=2,
                                       name=f"ps_p{j}_{tt}")
                        for hp in range(4):
                            nc.tensor.matmul(
                                ps_p,
                                lhsT=pwT8[:, 2 * hp:2 * hp + 2, j * 128:(j + 1) * 128],
                                rhs=a_sb[:, 2 * hp:2 * hp + 2, tt * 512:(tt + 1) * 512],
                                start=(hp == 0), stop=(hp == 3),
                                perf_mode=DR)
                        nc.vector.scalar_tensor_tensor(
                            out=out_t[:, tt * 512:(tt + 1) * 512], in0=ps_p,
                            scalar=1.0 / (WSCALE * WSCALE),
                            in1=x_sb[:, j, tt * 512:(tt + 1) * 512],
                            op0=mybir.AluOpType.mult, op1=mybir.AluOpType.add)
                    nc.scalar.dma_start(out=o_dv[:, j, :], in_=out_t)

    if split_mw:
        _split_multiwait(nc)
    return nc


_NC_CACHE = {}


def _get_program(repeat=1, loop_n=0):
    key = (repeat, loop_n)
    if key not in _NC_CACHE:
        _NC_CACHE[key] = _build_program(repeat, loop_n)
    return _NC_CACHE[key]


def _prep_shared(norm_w, norm_b, qkv_w, qkv_b, proj_w, proj_b):
    qkv_w = np.asarray(qkv_w, dtype=np.float32)
    proj_w = np.asarray(proj_w, dtype=np.float32)
    qkv_b = np.asarray(qkv_b, dtype=np.float32)
    proj_b = np.asarray(proj_b, dtype=np.float32)

    wq = qkv_w.reshape(3, NH, D, C)
    qb = qkv_b.reshape(3, NH, D)
    wqkv8 = np.zeros((CP, NCOLS), dtype=np.float32)
    for w in range(2):  # q, k: padded-to-128 head blocks
        for h in range(NH):
            base = w * QCOLS + h * DP
            wqkv8[0:C, base:base + D] = WSCALE * wq[w, h].T
            wqkv8[C, base:base + D] = WSCALE * qb[w, h]
    for h in range(NH):  # v: 80-wide head blocks
        base = VBASE + h * D
        wqkv8[0:C, base:base + D] = WSCALE * wq[2, h].T
        wqkv8[C, base:base + D] = WSCALE * qb[2, h]

    pwT8 = np.zeros((97, NH, C), dtype=np.float32)
    pwT8[0:D] = WSCALE * proj_w.reshape(C, NH, D).transpose(2, 1, 0)
    pwT8[96, 0, :] = WSCALE * proj_b

    cidx = np.arange(C) // GS
    ind1 = np.zeros((C, G), dtype=np.float32)
    ind1[np.arange(C), cidx] = 1.0
    ind2 = np.ascontiguousarray(ind1.T)
    return {
        "ind1": ind1,
        "ind2": ind2,
        "wqkv8": np.ascontiguousarray(wqkv8).astype(ml_dtypes.float8_e4m3),
        "pwT8": np.ascontiguousarray(pwT8).astype(ml_dtypes.float8_e4m3),
        "nw": np.ascontiguousarray(np.asarray(norm_w, dtype=np.float32)),
        "nb": np.ascontiguousarray(np.asarray(norm_b, dtype=np.float32)),
    }


def make_in_maps(x, norm_w, norm_b, qkv_w, qkv_b, proj_w, proj_b):
    x = np.asarray(x, dtype=np.float32)
    shared = _prep_shared(norm_w, norm_b, qkv_w, qkv_b, proj_w, proj_b)
    xs = x.reshape(B, C, T)
    return [dict(shared, x=np.ascontiguousarray(xs[i])) for i in range(B)]


def kernel(x, norm_w, norm_b, qkv_w, qkv_b, proj_w, proj_b):
    nc = _get_program()
    in_maps = make_in_maps(x, norm_w, norm_b, qkv_w, qkv_b, proj_w, proj_b)
    res = run_bass_kernel_spmd(nc, in_maps, core_ids=list(range(B)), trace=False)
    out = np.stack([res.results[i]["o"].reshape(C, HH, WW) for i in range(B)])
    return out.astype(np.float32)



# revision 21
# speedup vs baseline: 1.1432x; 1.1432x over previous
"""GAttentionBlock (GroupNorm + 8-head self-attention + proj + residual) on 8
Trainium2 NeuronCores, data-parallel over batch (B=8 -> 1 image per core).

v2: fp8(e4m3)+DoubleRow rework of the bf16 baseline. DoubleRow halves the
per-K-split cost wherever the contraction has >=2 128-tiles, so:
  - qkv: xn stored fp8 [128, 6, T] (C padded 640->768; row 640 is a constant-1
    bias row, chunk 5 otherwise zero), weights fp8 x16; 3 DR matmuls per
    512-half instead of 5 bf16 ones. q/k psum copied to bf16 [128(d pad), H, T]
    (d padded to 128 with zero weight cols so scores can contract K=128).
  - scores: single K-split (d=80) so DR cannot help; plain bf16 matmuls with
    FWL. exp on ACT with scale 1/(256*sqrt(80)) and bias -2 (keeps exp in
    e4m3 range; denominator comes from the same quantized probs so the shift
    cancels), probs written fp8.
  - AV: vT fp8 [128, H, 8sc, 112] (x16, ones col at 96 emits the softmax
    denominator at psum partition 96), probs fp8: 4 DR matmuls over sc pairs
    instead of 8.
  - proj: a_sb fp8 [81, H, T] (x16; row 80 constant 1), pwT fp8 [81, H, C]
    (x16; row 80 of head 0 carries 16*proj_b): 4 DR matmuls over head pairs;
    out = psum/256 + x fused in one DVE scalar_tensor_tensor.
All biases are folded into matmul rows; ACT runs only exp (the new bottleneck
engine); psum->sbuf copies are split DVE/GpSimd. Scaling: w,v,a,pw carry x16
into fp8 so ~N(0,1/C) weights land in e4m3's normal range; the net 256x on
scores/proj is divided back in the exp scale / output scale.
"""
import copy

import numpy as np
import ml_dtypes

import concourse.bass as bass
import concourse.mybir as mybir
import concourse.tile as tile
from concourse.bass_utils import run_bass_kernel_spmd

F32 = mybir.dt.float32
BF16 = mybir.dt.bfloat16
F8 = mybir.dt.float8e4

B, C, HH, WW = 8, 640, 32, 32
T = HH * WW            # 1024
NH, D = 8, 80          # heads, head dim
DP = 128               # padded head dim (q/k)
G = 32                 # groupnorm groups
GS = C // G            # 20 channels per group
EPS = 1e-5
NCHUNK = C // 128      # 5 channel chunks of 128
CP = 768               # padded channels (bias row at 640)
NCHP = CP // 128       # 6
NSC = T // 128         # 8 sequence chunks of 128
VW = 112               # vT row width (80 v + pad + ones col at 96)
QCOLS = NH * DP        # 1024
VBASE = 2 * QCOLS      # 2048
NCOLS = 2 * QCOLS + NH * D  # 2688
WSCALE = 16.0
EXPSHIFT = -2.0
EXPSCALE = 1.0 / (WSCALE * WSCALE * np.sqrt(np.float64(D)))

_MAXW = 1
GP_XN = False  # gpsimd tensor_scalar measured ~15us on HW (10x the model) — keep Q7 out of the loop

# sc-pair indices (of 4 per head) whose exp runs on DVE as a piecewise-linear
# bf16 bit-trick instead of ACT exp: bits = int16(128*log2e*(scale*x+shift)
# + 16319.6) reinterpreted as bf16 ~= exp(scale*x+shift) within +-3.5%.
# t is always in [14500, 17100] for any plausible score, so the int16
# convert can neither underflow nor overflow: no clamp needed.
DVE_PAIRS: dict[int, tuple[int, ...]] = {h: (3,) for h in range(NH)}
LOG2E = float(np.log2(np.e))

# AV-stage timing-attribution variants (hwav.py): A=DMA-bounce broadcast,
# B=PE-broadcast all heads, C=no denominator (timing-only, wrong numerics)
AV_VARIANT = "A"


def _split_multiwait(nc):
    """This walrus build rejects >1 sync-wait command per instruction. Move
    extra waits onto same-engine NoOps inserted just before the instruction."""
    ctr = 0
    new_module = copy.replace(nc.m, functions=[])
    for function in nc.m.functions:
        new_function = copy.replace(function, blocks=[])
        new_function.set_allocations_from_list(function.allocations)
        for block in function.blocks:
            new_insts = []
            for inst in block.instructions:
                si = inst.sync_info
                ow = list(si.on_wait) if (si is not None and si.on_wait) else []
                if len(ow) > _MAXW:
                    head, tail = ow[:-_MAXW], ow[-_MAXW:]
                    for w in head:
                        ctr += 1
                        new_insts.append(mybir.InstNoOp(
                            name=f"mwsplit_{ctr}",
                            engine=inst.engine,
                            sync_info=mybir.SyncInfo(on_wait=[w], on_update=[]),
                            bass_nofuse=True,
                        ))
                    inst.sync_info = mybir.SyncInfo(
                        on_wait=tail,
                        on_update=list(si.on_update) if si.on_update else [],
                    )
                new_insts.append(inst)
            new_function.blocks.append(copy.replace(block, instructions=new_insts))
        new_module.functions.append(new_function)
    nc.m = new_module


def _build_program(repeat=1, loop_n=0, split_mw=True, phase=5, no_init=False):
    # no_init: sim-only — skip the one-time gpsimd memsets (they live outside
    # the For_i loop on HW, so the per-iteration body never pays them)
    nc = bass.Bass("TRN2", target_bir_lowering=False, num_devices=8)
    DR = mybir.MatmulPerfMode.DoubleRow

    x_d = nc.dram_tensor("x", [C, T], F32, kind="ExternalInput").ap()
    wqkv_d = nc.dram_tensor("wqkv8", [CP, NCOLS], F8, kind="ExternalInput").ap()
    pwT_d = nc.dram_tensor("pwT8", [97, NH, C], F8, kind="ExternalInput").ap()
    nw_d = nc.dram_tensor("nw", [C], F32, kind="ExternalInput").ap()
    nb_d = nc.dram_tensor("nb", [C], F32, kind="ExternalInput").ap()
    ind1_d = nc.dram_tensor("ind1", [C, G], F32, kind="ExternalInput").ap()
    ind2_d = nc.dram_tensor("ind2", [G, C], F32, kind="ExternalInput").ap()
    o_d = nc.dram_tensor("o", [C, T], F32, kind="ExternalOutput").ap()

    x_dv = x_d.rearrange("(o p) t -> p o t", p=128)       # [128, 5, 1024]
    o_dv = o_d.rearrange("(o p) t -> p o t", p=128)

    with tile.TileContext(nc) as tc:
        with tc.tile_pool(name="wpool", bufs=1) as wp, \
             tc.tile_pool(name="data", bufs=1) as dp, \
             tc.tile_pool(name="ptile", bufs=2) as pp, \
             tc.tile_pool(name="small", bufs=2) as sp, \
             tc.tile_pool(name="ps", bufs=2, space="PSUM") as ps, \
             tc.tile_pool(name="dram", bufs=2, space="DRAM") as dr:

            # ---------- weight / constant loads (ACT DMA queue: keeps the
            # SP queue free for the per-iteration x loads) ----------
            nwb = wp.tile([128, NCHUNK, 2], F32)
            nc.scalar.dma_start(out=nwb[:, :, 0], in_=nw_d.rearrange("(o p) -> p o", p=128))
            nc.scalar.dma_start(out=nwb[:, :, 1], in_=nb_d.rearrange("(o p) -> p o", p=128))

            ind1 = wp.tile([128, NCHUNK, G], F32)   # [channel -> group] one-hot
            ind2 = wp.tile([G, NCHUNK, 128], F32)   # [group -> channel] one-hot
            nc.scalar.dma_start(out=ind1, in_=ind1_d.rearrange("(o p) g -> p o g", p=128))
            nc.scalar.dma_start(out=ind2, in_=ind2_d.rearrange("g (o p) -> g o p", p=128))

            wqkv8 = wp.tile([128, NCHP, NCOLS], F8)
            nc.scalar.dma_start(out=wqkv8, in_=wqkv_d.rearrange("(o p) n -> p o n", p=128))
            pwT8 = wp.tile([97, NH, C], F8)
            nc.scalar.dma_start(out=pwT8, in_=pwT_d)

            eps_t = wp.tile([G, 1], F32)
            nc.vector.memset(eps_t, EPS)
            neg2 = wp.tile([128, 1], F32)
            nc.vector.memset(neg2, EXPSHIFT)
            ones80 = wp.tile([1, D], F32)
            nc.vector.memset(ones80, 1.0)

            # ---------- persistent data tiles + one-time inits ----------
            # x/xn double-buffered across repeat-bodies so body k+1's input
            # load + GN overlap body k's attention phase
            nxb = min(repeat, 2)
            xn_bufs, x_bufs = [], []
            for bi in range(nxb):
                xnb = dp.tile([128, NCHP, T], F8, name=f"xn{bi}")
                nc.gpsimd.memset(xnb[:, NCHP - 1, :], 0.0)
                nc.gpsimd.memset(xnb[0:1, NCHP - 1, :], 1.0)  # bias row 640
                xn_bufs.append(xnb)
                x_bufs.append(dp.tile([128, NCHUNK, T], F32, name=f"x_sb{bi}"))
            vT = dp.tile([128, NH, NSC, VW], F8)
            nc.gpsimd.memset(vT[:, :, :, D:VW], 0.0)
            nc.gpsimd.memset(vT[:, :, :, 96:97], 1.0)     # denominator ones col
            a_sb = dp.tile([97, NH, T], F8)
            nc.gpsimd.memset(a_sb[64:97, :, :], 0.0)
            nc.gpsimd.memset(a_sb[96:97, :, :], 1.0)      # proj bias row
            q_sb = dp.tile([128, NH, T], BF16)
            k_sb = dp.tile([128, NH, T], BF16)

            import contextlib
            loop_cm = tc.For_i(0, loop_n, 1) if loop_n else contextlib.nullcontext()
            with loop_cm:
              for _rep in range(repeat):
                xn = xn_bufs[_rep % nxb]
                x_sb = x_bufs[_rep % nxb]
                # ---------- stage A: load x + GroupNorm ----------
                # split across two DMA queues so the head isn't serialized
                # behind a single queue's 5-chunk stream
                for j in range(NCHUNK):
                    eng = nc.sync if j % 2 == 0 else nc.scalar
                    eng.dma_start(out=x_sb[:, j, :], in_=x_dv[:, j, :])

                stats = sp.tile([128, 2, 6], F32, tag="gn_stats")
                ss = sp.tile([128, NCHUNK, 2], F32, tag="gn_ss")
                for j in range(NCHUNK):
                    nc.vector.bn_stats(out=stats[:, 0, :], in_=x_sb[:, j, 0:512])
                    nc.vector.bn_stats(out=stats[:, 1, :], in_=x_sb[:, j, 512:1024])
                    nc.vector.bn_aggr(out=ss[:, j, :], in_=stats)
                    # ss[...,1] currently var; make it var + mean^2 = E[x^2]
                    nc.vector.scalar_tensor_tensor(
                        out=ss[:, j, 1:2], in0=ss[:, j, 0:1],
                        scalar=ss[:, j, 0:1], in1=ss[:, j, 1:2],
                        op0=mybir.AluOpType.mult, op1=mybir.AluOpType.add)

                ps_g = ps.tile([G, 2], F32, tag="work", bufs=2)
                for j in range(NCHUNK):
                    nc.tensor.matmul(ps_g, lhsT=ind1[:, j, :], rhs=ss[:, j, :],
                                     start=(j == 0), stop=(j == NCHUNK - 1))
                # group stats -> mean_g, rstd_g
                gm = sp.tile([G, 2], F32, tag="gn_gm")       # [mean_g, rstd_g]
                tmp_g = sp.tile([G, 2], F32, tag="gn_tmp")
                nc.vector.tensor_scalar_mul(gm, ps_g, 1.0 / GS)           # [mean, E2]
                nc.vector.tensor_tensor(out=tmp_g[:, 0:1], in0=gm[:, 0:1],
                                        in1=gm[:, 0:1], op=mybir.AluOpType.mult)
                nc.vector.tensor_tensor(out=tmp_g[:, 1:2], in0=gm[:, 1:2],
                                        in1=tmp_g[:, 0:1], op=mybir.AluOpType.subtract)
                nc.scalar.activation(out=tmp_g[:, 1:2], in_=tmp_g[:, 1:2],
                                     func=mybir.ActivationFunctionType.Ln,
                                     bias=eps_t, scale=1.0)
                nc.scalar.activation(out=gm[:, 1:2], in_=tmp_g[:, 1:2],
                                     func=mybir.ActivationFunctionType.Exp,
                                     scale=-0.5)   # rstd_g = (var+eps)^-0.5

                ab = sp.tile([128, NCHUNK, 2], F32, tag="gn_ab")
                for j in range(NCHUNK):
                    ps_bc = ps.tile([128, 2], F32, tag="work", bufs=2, name=f"ps_bc{j}")
                    nc.tensor.matmul(ps_bc, lhsT=ind2[:, j, :], rhs=gm,
                                     start=True, stop=True)
                    # A = rstd_c * norm_w ; B = norm_b - mean_c * A
                    nc.vector.tensor_tensor(out=ab[:, j, 0:1], in0=ps_bc[:, 1:2],
                                            in1=nwb[:, j, 0:1], op=mybir.AluOpType.mult)
                    nc.vector.scalar_tensor_tensor(
                        out=stats[:, 0, 0:1], in0=ps_bc[:, 0:1],
                        scalar=ab[:, j, 0:1], in1=nwb[:, j, 1:2],
                        op0=mybir.AluOpType.mult, op1=mybir.AluOpType.subtract)
                    nc.vector.tensor_scalar_mul(ab[:, j, 1:2],
                                                stats[:, 0, 0:1], -1.0)
                    if j % 2 == 0:
                        # ACT is idle during the GN head: apply xn there
                        nc.scalar.activation(
                            out=xn[:, j, :], in_=x_sb[:, j, :],
                            func=mybir.ActivationFunctionType.Identity,
                            bias=ab[:, j, 1:2], scale=ab[:, j, 0:1])
                    else:
                        nc.vector.tensor_scalar(
                            out=xn[:, j, :], in0=x_sb[:, j, :],
                            scalar1=ab[:, j, 0:1], scalar2=ab[:, j, 1:2],
                            op0=mybir.AluOpType.mult,
                            op1=mybir.AluOpType.add)

                # ---------- stages B+C: software-pipelined qkv + attention ----
                # PE is in-order and co-saturated with ACT, so each head
                # iteration interleaves stall-free "filler" matmul steps
                # (qk(h+1), AV(h-1), vT chains) between the scores pairs,
                # which are the only PE ops that wait on ACT (psum reuse).
                # Chains never span an iteration (pool round-robin + in-order
                # PE would deadlock); fillers are emitted chain-by-chain.
                p_tiles = {}

                def qk_steps(h):
                    steps = []
                    for w in range(2):  # 0=q, 1=k
                        dst = q_sb if w == 0 else k_sb
                        for tt in range(2):
                            cell = {}

                            def step(jp, w=w, tt=tt, dst=dst, cell=cell, h=h):
                                if jp == 0:
                                    cell['ps'] = ps.tile(
                                        [128, 512], F32, tag="work", bufs=2,
                                        name=f"ps_qk{h}_{w}_{tt}")
                                nc.tensor.matmul(
                                    cell['ps'],
                                    lhsT=wqkv8[:, 2 * jp:2 * jp + 2,
                                               w * QCOLS + h * DP:
                                               w * QCOLS + (h + 1) * DP],
                                    rhs=xn[:, 2 * jp:2 * jp + 2,
                                           tt * 512:(tt + 1) * 512],
                                    start=(jp == 0), stop=(jp == 2),
                                    perf_mode=DR)
                                if jp == 2:
                                    nc.vector.tensor_copy(
                                        out=dst[:, h, tt * 512:(tt + 1) * 512],
                                        in_=cell['ps'])
                            steps += [lambda jp=jp, s=step: s(jp)
                                      for jp in range(3)]
                    return steps

                def vT_steps(nn, scs):  # nn: 4-head group; scs: sc chunks
                    steps = []
                    for sc in scs:
                        cell = {}

                        def step(jp, sc=sc, nn=nn, cell=cell):
                            if jp == 0:
                                cell['ps'] = ps.tile(
                                    [128, 4 * D], F32, tag="work", bufs=2,
                                    name=f"ps_v{sc}_{nn}")
                            nc.tensor.matmul(
                                cell['ps'],
                                lhsT=xn[:, 2 * jp:2 * jp + 2,
                                        sc * 128:(sc + 1) * 128],
                                rhs=wqkv8[:, 2 * jp:2 * jp + 2,
                                          VBASE + nn * 4 * D:
                                          VBASE + (nn + 1) * 4 * D],
                                start=(jp == 0), stop=(jp == 2),
                                perf_mode=DR)
                            if jp == 2:
                                nc.vector.tensor_copy(
                                    out=vT[:, nn * 4:(nn + 1) * 4, sc, 0:D],
                                    in_=cell['ps'].rearrange(
                                        "p (h d) -> p h d", h=4))
                        steps += [lambda jp=jp, s=step: s(jp) for jp in range(3)]
                    return steps

                def av_parts(h, tag="av"):
                    """AV chains + softmax-denominator trailer for head h.

                    Returns (chain, trailers): chain[tt][scp] and trailers[tt]
                    as zero-arg fns. chain steps honor DVE_PAIRS: offloaded
                    pairs read the bf16 probs tile with two plain matmuls
                    instead of one fp8 DoubleRow matmul.

                    The denominator uses reciprocal_approx_fast (fp32, ~18
                    good bits — InstReciprocal measured ~5x slower on HW) and
                    the trailer multiplies are deferred a full head so the
                    ~3us DRAM-bounce broadcast latency stays off the DVE
                    critical path."""
                    p_t, p_bf, dvp = p_tiles.pop(h)
                    rinv = sp.tile([1, T], F32, tag="rinv", name=f"rinv{h}")
                    rb = sp.tile([D, T], F32, tag="rb", name=f"rb{h}")
                    r_dr = dr.tile([1, T], F32, tag="rbounce", name=f"r_dr{h}")
                    cells = [{}, {}]
                    chain = [[], []]
                    for tt in range(2):
                        sl = slice(tt * 512, (tt + 1) * 512)
                        cell = cells[tt]

                        def step(scp, sl=sl, cell=cell, h=h, p_t=p_t,
                                 p_bf=p_bf, dvp=dvp, rinv=rinv, rb=rb,
                                 r_dr=r_dr, tt=tt, tag=tag):
                            if scp == 0:
                                cell['ps'] = ps.tile(
                                    [VW, 512], F32, tag=tag,
                                    name=f"ps_a{h}_{tt}", bufs=2)
                            ps_a = cell['ps']
                            if scp in dvp:
                                pi = dvp.index(scp)
                                for sub in range(2):
                                    nc.tensor.matmul(
                                        ps_a,
                                        lhsT=vT[:, h, 2 * scp + sub, :],
                                        rhs=p_bf[:, pi, sub, sl],
                                        start=(scp == 0 and sub == 0),
                                        stop=(scp == 3 and sub == 1))
                            else:
                                nc.tensor.matmul(
                                    ps_a,
                                    lhsT=vT[:, h, 2 * scp:2 * scp + 2, :],
                                    rhs=p_t[:, 2 * scp:2 * scp + 2, sl],
                                    start=(scp == 0), stop=(scp == 3),
                                    perf_mode=DR)
                            if scp == 3 and AV_VARIANT != "C":
                                # 1/d = exp(-ln d): both in ACT's exp table
                                # set, so no table reload amid the probs exp
                                # stream. (InstReciprocal on DVE measured
                                # ~2.7us per [1,512] — 43us/iter — and the
                                # custom-DVE approx ops don't compile here.)
                                lnd = sp.tile([1, T], F32, tag="lnd",
                                              name=f"lnd{h}")
                                nc.scalar.activation(
                                    out=lnd[0:1, sl], in_=ps_a[96:97, :],
                                    func=mybir.ActivationFunctionType.Ln,
                                    scale=1.0)
                                nc.scalar.activation(
                                    out=rinv[0:1, sl], in_=lnd[0:1, sl],
                                    func=mybir.ActivationFunctionType.Exp,
                                    scale=-1.0)
                                if h != NH - 1:
                                    # launch the DRAM-bounce broadcast now;
                                    # the consuming multiply runs a head
                                    # later, hiding the ~3us round trip
                                    nc.sync.dma_start(out=r_dr[:, sl],
                                                      in_=rinv[:, sl])
                                    nc.sync.dma_start(
                                        out=rb[:, sl],
                                        in_=r_dr[0:1, sl].to_broadcast(
                                            [D, 512]))
                        chain[tt] = [lambda scp=scp, s=step: s(scp)
                                     for scp in range(4)]

                    def trailer(tt, h=h, rinv=rinv, rb=rb, cells=cells):
                        sl = slice(tt * 512, (tt + 1) * 512)
                        if AV_VARIANT == "C":
                            nc.vector.tensor_copy(
                                out=a_sb[0:D, h, sl],
                                in_=cells[tt]['ps'][0:D, :])
                            return
                        if h == NH - 1:
                            # PE-broadcast + ACT evacuation (low latency).
                            # psum tag: "av"/"work" hold live AV cells here —
                            # allocating them would deadlock pool rotation.
                            ps_rb = ps.tile([D, 512], F32, tag="scores",
                                            bufs=2, name=f"ps_rb{h}_{tt}")
                            nc.tensor.matmul(ps_rb, lhsT=ones80,
                                             rhs=rinv[0:1, sl],
                                             start=True, stop=True)
                            nc.scalar.copy(out=rb[:, sl], in_=ps_rb)
                        nc.vector.tensor_tensor(
                            out=a_sb[0:D, h, sl], in0=cells[tt]['ps'][0:D, :],
                            in1=rb[:, sl], op=mybir.AluOpType.mult)

                    return chain, [lambda: trailer(0), lambda: trailer(1)]

                def run_iter(h, fillers, post=()):
                    """post: list of (sc_pos, [fns]) run right after sc_pos's
                    exp is emitted (sc_pos=NSC: after the loop). Positioned a
                    slot late by callers so the next scores pair is already
                    in the PE queue and ACT never bubbles."""
                    dvp = tuple(DVE_PAIRS.get(h, ()))
                    p_t = pp.tile([128, NSC, T], F8, tag="probs", name=f"p_t{h}")
                    p_bf = None
                    if dvp:
                        p_bf = pp.tile([128, len(dvp), 2, T], BF16,
                                       tag="pbf", name=f"p_bf{h}")
                    p_tiles[h] = (p_t, p_bf, dvp)
                    post_map = {}
                    for pos, fns in post:
                        post_map.setdefault(pos, []).extend(fns)
                    nf = len(fillers)
                    done = 0
                    for sc in range(NSC):
                        want = (sc * nf) // NSC
                        while done < want:
                            fillers[done]()
                            done += 1
                        ps_s = ps.tile([128, T], F32, tag="scores",
                                       name=f"ps_s{h}_{sc}", bufs=2)
                        for tt in range(2):
                            nc.tensor.matmul(
                                ps_s[:, tt * 512:(tt + 1) * 512],
                                lhsT=k_sb[:, h, sc * 128:(sc + 1) * 128],
                                rhs=q_sb[:, h, tt * 512:(tt + 1) * 512],
                                start=True, stop=True)
                        if sc // 2 in dvp:
                            pi = dvp.index(sc // 2)
                            nc.vector.tensor_scalar(
                                out=p_bf[:, pi, sc % 2, :].bitcast(
                                    mybir.dt.int16),
                                in0=ps_s,
                                scalar1=float(128.0 * LOG2E * EXPSCALE),
                                scalar2=float(16256.0 + 128.0 * LOG2E
                                              * EXPSHIFT - 5.0),
                                op0=mybir.AluOpType.mult,
                                op1=mybir.AluOpType.add)
                        else:
                            nc.scalar.activation(
                                out=p_t[:, sc, :], in_=ps_s,
                                func=mybir.ActivationFunctionType.Exp,
                                bias=neg2, scale=float(EXPSCALE))
                        for fn in post_map.pop(sc, ()):
                            fn()
                    while done < nf:
                        fillers[done]()
                        done += 1
                    for fn in post_map.pop(NSC, ()):
                        fn()

                if phase == 2:
                    # timing-attribution subset: all qkv/vT chains, no attn
                    for h in range(NH):
                        for s in qk_steps(h):
                            s()
                    for s in vT_steps(0, range(NSC)) + vT_steps(1, range(NSC)):
                        s()
                elif phase == 3:
                    for s in qk_steps(0):
                        s()
                    run_iter(0, qk_steps(1) + vT_steps(0, range(NSC)))
                    nn1 = vT_steps(1, range(NSC))
                    for h in range(1, NH):
                        fillers = []
                        if h + 1 < NH:
                            fillers += qk_steps(h + 1)
                        if 1 <= h <= 4:
                            fillers += nn1[(h - 1) * 6:h * 6]
                        run_iter(h, fillers)
                        p_tiles.pop(h - 1, None)
                elif phase >= 4:
                    for s in qk_steps(0):
                        s()
                    run_iter(0, qk_steps(1) + vT_steps(0, range(NSC)))
                    nn1 = vT_steps(1, range(NSC))  # 24 steps: spread h=1..4
                    trail = {}  # h -> deferred trailer fns
                    for h in range(1, NH):
                        fillers = []
                        if h + 1 < NH:
                            fillers += qk_steps(h + 1)
                        if h - 2 in trail:
                            # trailer multiplies deferred one head: rb(h-2)'s
                            # DRAM bounce has had a full head to land
                            fillers += trail.pop(h - 2)
                        chain, trailers = av_parts(h - 1)
                        fillers += chain[0] + chain[1]
                        trail[h - 1] = trailers
                        post = ()
                        if h == NH - 1:
                            # last head: AV(7) rides inside the exp window,
                            # positioned a slot after its probs pair; its
                            # psum uses the now-idle "work" tag so it never
                            # contends with AV(6) in the "av" tag
                            deferred = {}

                            def defer(idx, tt=None, h=h):
                                def go():
                                    if 'c' not in deferred:
                                        deferred['c'], deferred['t'] = \
                                            av_parts(h, tag="work")
                                    if tt is None:
                                        for t2 in range(2):
                                            deferred['c'][t2][idx]()
                                    else:
                                        deferred['t'][tt]()
                                return go

                            post = [(2, [defer(0)]), (4, [defer(1)]),
                                    (6, [defer(2)]),
                                    (NSC, [defer(3), defer(None, 0),
                                           defer(None, 1)])]
                        if 1 <= h <= 4:
                            fillers += nn1[(h - 1) * 6:h * 6]
                        run_iter(h, fillers, post=post)
                    for fn in trail.pop(NH - 2):  # trailers(6) at tail
                        fn()

                # ---------- stage D: proj + bias + residual ----------
                for j in range(NCHUNK if phase >= 5 else 0):
                    out_t = sp.tile([128, T], F32, tag="out", name=f"out{j}")
                    for tt in range(2):
                        ptag = ("work", "scores", "av")[(2 * j + tt) % 3]
                        ps_p = ps.tile([128, 512], F32, tag=ptag,
                                       bufs=2,
                                       name=f"ps_p{j}_{tt}")
                        for hp in range(4):
                            nc.tensor.matmul(
                                ps_p,
                                lhsT=pwT8[:, 2 * hp:2 * hp + 2, j * 128:(j + 1) * 128],
                                rhs=a_sb[:, 2 * hp:2 * hp + 2, tt * 512:(tt + 1) * 512],
                                start=(hp == 0), stop=(hp == 3),
                                perf_mode=DR)
                        nc.vector.scalar_tensor_tensor(
                            out=out_t[:, tt * 512:(tt + 1) * 512], in0=ps_p,
                            scalar=1.0 / (WSCALE * WSCALE),
                            in1=x_sb[:, j, tt * 512:(tt + 1) * 512],
                            op0=mybir.AluOpType.mult, op1=mybir.AluOpType.add)
                    nc.scalar.dma_start(out=o_dv[:, j, :], in_=out_t)

    if split_mw:
        _split_multiwait(nc)
    return nc


_NC_CACHE = {}


def _get_program(repeat=1, loop_n=0):
    key = (repeat, loop_n)
    if key not in _NC_CACHE:
        _NC_CACHE[key] = _build_program(repeat, loop_n)
    return _NC_CACHE[key]


def _prep_shared(norm_w, norm_b, qkv_w, qkv_b, proj_w, proj_b):
    qkv_w = np.asarray(qkv_w, dtype=np.float32)
    proj_w = np.asarray(proj_w, dtype=np.float32)
    qkv_b = np.asarray(qkv_b, dtype=np.float32)
    proj_b = np.asarray(proj_b, dtype=np.float32)

    wq = qkv_w.reshape(3, NH, D, C)
    qb = qkv_b.reshape(3, NH, D)
    wqkv8 = np.zeros((CP, NCOLS), dtype=np.float32)
    for w in range(2):  # q, k: padded-to-128 head blocks
        for h in range(NH):
            base = w * QCOLS + h * DP
            wqkv8[0:C, base:base + D] = WSCALE * wq[w, h].T
            wqkv8[C, base:base + D] = WSCALE * qb[w, h]
    for h in range(NH):  # v: 80-wide head blocks
        base = VBASE + h * D
        wqkv8[0:C, base:base + D] = WSCALE * wq[2, h].T
        wqkv8[C, base:base + D] = WSCALE * qb[2, h]

    pwT8 = np.zeros((97, NH, C), dtype=np.float32)
    pwT8[0:D] = WSCALE * proj_w.reshape(C, NH, D).transpose(2, 1, 0)
    pwT8[96, 0, :] = WSCALE * proj_b

    cidx = np.arange(C) // GS
    ind1 = np.zeros((C, G), dtype=np.float32)
    ind1[np.arange(C), cidx] = 1.0
    ind2 = np.ascontiguousarray(ind1.T)
    return {
        "ind1": ind1,
        "ind2": ind2,
        "wqkv8": np.ascontiguousarray(wqkv8).astype(ml_dtypes.float8_e4m3),
        "pwT8": np.ascontiguousarray(pwT8).astype(ml_dtypes.float8_e4m3),
        "nw": np.ascontiguousarray(np.asarray(norm_w, dtype=np.float32)),
        "nb": np.ascontiguousarray(np.asarray(norm_b, dtype=np.float32)),
    }


def make_in_maps(x, norm_w, norm_b, qkv_w, qkv_b, proj_w, proj_b):
    x = np.asarray(x, dtype=np.float32)
    shared = _prep_shared(norm_w, norm_b, qkv_w, qkv_b, proj_w, proj_b)
    xs = x.reshape(B, C, T)
    return [dict(shared, x=np.ascontiguousarray(xs[i])) for i in range(B)]


def kernel(x, norm_w, norm_b, qkv_w, qkv_b, proj_w, proj_b):
    nc = _get_program()
    in_maps = make_in_maps(x, norm_w, norm_b, qkv_w, qkv_b, proj_w, proj_b)
    res = run_bass_kernel_spmd(nc, in_maps, core_ids=list(range(B)), trace=False)
    out = np.stack([res.results[i]["o"].reshape(C, HH, WW) for i in range(B)])
    return out.astype(np.float32)



# revision 26
# speedup vs baseline: 1.2676x; 1.1089x over previous
"""GAttentionBlock (GroupNorm + 8-head self-attention + proj + residual) on 8
Trainium2 NeuronCores, data-parallel over batch (B=8 -> 1 image per core).

v2: fp8(e4m3)+DoubleRow rework of the bf16 baseline. DoubleRow halves the
per-K-split cost wherever the contraction has >=2 128-tiles, so:
  - qkv: xn stored fp8 [128, 6, T] (C padded 640->768; row 640 is a constant-1
    bias row, chunk 5 otherwise zero), weights fp8 x16; 3 DR matmuls per
    512-half instead of 5 bf16 ones. q/k psum copied to bf16 [128(d pad), H, T]
    (d padded to 128 with zero weight cols so scores can contract K=128).
  - scores: single K-split (d=80) so DR cannot help; plain bf16 matmuls with
    FWL. exp on ACT with scale 1/(256*sqrt(80)) and bias -2 (keeps exp in
    e4m3 range; denominator comes from the same quantized probs so the shift
    cancels), probs written fp8.
  - AV: vT fp8 [128, H, 8sc, 112] (x16, ones col at 96 emits the softmax
    denominator at psum partition 96), probs fp8: 4 DR matmuls over sc pairs
    instead of 8.
  - proj: a_sb fp8 [81, H, T] (x16; row 80 constant 1), pwT fp8 [81, H, C]
    (x16; row 80 of head 0 carries 16*proj_b): 4 DR matmuls over head pairs;
    out = psum/256 + x fused in one DVE scalar_tensor_tensor.
All biases are folded into matmul rows; ACT runs only exp (the new bottleneck
engine); psum->sbuf copies are split DVE/GpSimd. Scaling: w,v,a,pw carry x16
into fp8 so ~N(0,1/C) weights land in e4m3's normal range; the net 256x on
scores/proj is divided back in the exp scale / output scale.
"""
import copy

import numpy as np
import ml_dtypes

import concourse.bass as bass
import concourse.mybir as mybir
import concourse.tile as tile
from concourse.bass_utils import run_bass_kernel_spmd

F32 = mybir.dt.float32
BF16 = mybir.dt.bfloat16
F8 = mybir.dt.float8e4

B, C, HH, WW = 8, 640, 32, 32
T = HH * WW            # 1024
NH, D = 8, 80          # heads, head dim
DP = 128               # padded head dim (q/k)
G = 32                 # groupnorm groups
GS = C // G            # 20 channels per group
EPS = 1e-5
NCHUNK = C // 128      # 5 channel chunks of 128
CP = 768               # padded channels (bias row at 640)
NCHP = CP // 128       # 6
NSC = T // 128         # 8 sequence chunks of 128
VW = 112               # vT row width (80 v + pad + ones col at 96)
QCOLS = NH * DP        # 1024
VBASE = 2 * QCOLS      # 2048
NCOLS = 2 * QCOLS + NH * D  # 2688
WSCALE = 16.0
EXPSHIFT = -2.0
EXPSCALE = 1.0 / (WSCALE * WSCALE * np.sqrt(np.float64(D)))

_MAXW = 1
GP_XN = False  # gpsimd tensor_scalar measured ~15us on HW (10x the model) — keep Q7 out of the loop

# sc-pair indices (of 4 per head) whose exp runs on DVE as a piecewise-linear
# bf16 bit-trick instead of ACT exp: bits = int16(128*log2e*(scale*x+shift)
# + 16319.6) reinterpreted as bf16 ~= exp(scale*x+shift) within +-3.5%.
# t is always in [14500, 17100] for any plausible score, so the int16
# convert can neither underflow nor overflow: no clamp needed.
# pair 1 (mid-head): an end-of-head pair would gate the next head's first
# scores on the DVE queue draining, bubbling ACT ~1.8us per head
DVE_PAIRS: dict[int, tuple[int, ...]] = {h: (1,) for h in range(NH)}
LOG2E = float(np.log2(np.e))

# AV-stage timing-attribution variants (hwav.py): A=DMA-bounce broadcast,
# B=PE-broadcast all heads, C=no denominator (timing-only, wrong numerics)
AV_VARIANT = "A"


def _split_multiwait(nc):
    """This walrus build rejects >1 sync-wait command per instruction. Move
    extra waits onto same-engine NoOps inserted just before the instruction."""
    ctr = 0
    new_module = copy.replace(nc.m, functions=[])
    for function in nc.m.functions:
        new_function = copy.replace(function, blocks=[])
        new_function.set_allocations_from_list(function.allocations)
        for block in function.blocks:
            new_insts = []
            for inst in block.instructions:
                si = inst.sync_info
                ow = list(si.on_wait) if (si is not None and si.on_wait) else []
                if len(ow) > _MAXW:
                    head, tail = ow[:-_MAXW], ow[-_MAXW:]
                    for w in head:
                        ctr += 1
                        new_insts.append(mybir.InstNoOp(
                            name=f"mwsplit_{ctr}",
                            engine=inst.engine,
                            sync_info=mybir.SyncInfo(on_wait=[w], on_update=[]),
                            bass_nofuse=True,
                        ))
                    inst.sync_info = mybir.SyncInfo(
                        on_wait=tail,
                        on_update=list(si.on_update) if si.on_update else [],
                    )
                new_insts.append(inst)
            new_function.blocks.append(copy.replace(block, instructions=new_insts))
        new_module.functions.append(new_function)
    nc.m = new_module


def _build_program(repeat=1, loop_n=0, split_mw=True, phase=5, no_init=False):
    # no_init: sim-only — skip the one-time gpsimd memsets (they live outside
    # the For_i loop on HW, so the per-iteration body never pays them)
    nc = bass.Bass("TRN2", target_bir_lowering=False, num_devices=8)
    DR = mybir.MatmulPerfMode.DoubleRow

    x_d = nc.dram_tensor("x", [C, T], F32, kind="ExternalInput").ap()
    wqkv_d = nc.dram_tensor("wqkv8", [CP, NCOLS], F8, kind="ExternalInput").ap()
    pwT_d = nc.dram_tensor("pwT8", [97, NH, C], F8, kind="ExternalInput").ap()
    nw_d = nc.dram_tensor("nw", [C], F32, kind="ExternalInput").ap()
    nb_d = nc.dram_tensor("nb", [C], F32, kind="ExternalInput").ap()
    ind1_d = nc.dram_tensor("ind1", [C, G], F32, kind="ExternalInput").ap()
    ind2_d = nc.dram_tensor("ind2", [G, C], F32, kind="ExternalInput").ap()
    o_d = nc.dram_tensor("o", [C, T], F32, kind="ExternalOutput").ap()

    x_dv = x_d.rearrange("(o p) t -> p o t", p=128)       # [128, 5, 1024]
    o_dv = o_d.rearrange("(o p) t -> p o t", p=128)

    with tile.TileContext(nc) as tc:
        with tc.tile_pool(name="wpool", bufs=1) as wp, \
             tc.tile_pool(name="data", bufs=1) as dp, \
             tc.tile_pool(name="ptile", bufs=2) as pp, \
             tc.tile_pool(name="small", bufs=2) as sp, \
             tc.tile_pool(name="ps", bufs=2, space="PSUM") as ps, \
             tc.tile_pool(name="dram", bufs=2, space="DRAM") as dr:

            # ---------- weight / constant loads (ACT DMA queue: keeps the
            # SP queue free for the per-iteration x loads) ----------
            nwb = wp.tile([128, NCHUNK, 2], F32)
            ind1 = wp.tile([128, NCHUNK, G], F32)   # [channel -> group] one-hot
            ind2 = wp.tile([G, NCHUNK, 128], F32)   # [group -> channel] one-hot
            wqkv8 = wp.tile([128, NCHP, NCOLS], F8)
            pwT8 = wp.tile([97, NH, C], F8)
            eps_t = wp.tile([G, 1], F32)
            neg2 = wp.tile([128, 1], F32)
            ones80 = wp.tile([1, D], F32)
            if not no_init:
                nc.scalar.dma_start(out=nwb[:, :, 0], in_=nw_d.rearrange("(o p) -> p o", p=128))
                nc.scalar.dma_start(out=nwb[:, :, 1], in_=nb_d.rearrange("(o p) -> p o", p=128))
                nc.scalar.dma_start(out=ind1, in_=ind1_d.rearrange("(o p) g -> p o g", p=128))
                nc.scalar.dma_start(out=ind2, in_=ind2_d.rearrange("g (o p) -> g o p", p=128))
                nc.scalar.dma_start(out=wqkv8, in_=wqkv_d.rearrange("(o p) n -> p o n", p=128))
                nc.scalar.dma_start(out=pwT8, in_=pwT_d)
                nc.vector.memset(eps_t, EPS)
                nc.vector.memset(neg2, EXPSHIFT)
                nc.vector.memset(ones80, 1.0)
            else:
                # sim-only: tiny touch-writes so Tile sees every one-time
                # tile allocated without the 18us of Pool memsets
                for _t in (nwb[0:1, 0:1, 0:1], ind1[0:1, 0:1, 0:1],
                           ind2[0:1, 0:1, 0:1], wqkv8[0:1, 0:1, 0:1],
                           pwT8[0:1, 0:1, 0:1], eps_t[0:1, 0:1],
                           neg2[0:1, 0:1], ones80[0:1, 0:1]):
                    nc.gpsimd.memset(_t, 0.0)

            # ---------- persistent data tiles + one-time inits ----------
            # x/xn double-buffered across repeat-bodies so body k+1's input
            # load + GN overlap body k's attention phase
            nxb = min(repeat, 2)
            xn_bufs, x_bufs = [], []
            for bi in range(nxb):
                xnb = dp.tile([128, NCHP, T], F8, name=f"xn{bi}")
                if not no_init:
                    nc.gpsimd.memset(xnb[:, NCHP - 1, :], 0.0)
                    nc.gpsimd.memset(xnb[0:1, NCHP - 1, :], 1.0)  # bias row 640
                else:
                    nc.gpsimd.memset(xnb[0:1, NCHP - 1, 0:1], 1.0)
                xn_bufs.append(xnb)
                x_bufs.append(dp.tile([128, NCHUNK, T], F32, name=f"x_sb{bi}"))
            vT = dp.tile([128, NH, NSC, VW], F8)
            a_sb = dp.tile([97, NH, T], F8)
            if not no_init:
                nc.gpsimd.memset(vT[:, :, :, D:VW], 0.0)
                nc.gpsimd.memset(vT[:, :, :, 96:97], 1.0)     # denominator ones col
                nc.gpsimd.memset(a_sb[64:97, :, :], 0.0)
                nc.gpsimd.memset(a_sb[96:97, :, :], 1.0)      # proj bias row
            else:
                nc.gpsimd.memset(vT[0:1, 0:1, 0:1, 0:1], 1.0)
                nc.gpsimd.memset(a_sb[0:1, 0:1, 0:1], 1.0)
            q_sb = dp.tile([128, NH, T], BF16)
            k_sb = dp.tile([128, NH, T], BF16)

            import contextlib
            loop_cm = tc.For_i(0, loop_n, 1) if loop_n else contextlib.nullcontext()
            with loop_cm:
              for _rep in range(repeat):
                xn = xn_bufs[_rep % nxb]
                x_sb = x_bufs[_rep % nxb]
                # ---------- stage A: load x + GroupNorm ----------
                # split across two DMA queues so the head isn't serialized
                # behind a single queue's 5-chunk stream
                for j in range(NCHUNK):
                    eng = nc.sync if j % 2 == 0 else nc.scalar
                    eng.dma_start(out=x_sb[:, j, :], in_=x_dv[:, j, :])

                stats = sp.tile([128, 2, 6], F32, tag="gn_stats")
                ss = sp.tile([128, NCHUNK, 2], F32, tag="gn_ss")
                for j in range(NCHUNK):
                    nc.vector.bn_stats(out=stats[:, 0, :], in_=x_sb[:, j, 0:512])
                    nc.vector.bn_stats(out=stats[:, 1, :], in_=x_sb[:, j, 512:1024])
                    nc.vector.bn_aggr(out=ss[:, j, :], in_=stats)
                    # ss[...,1] currently var; make it var + mean^2 = E[x^2]
                    nc.vector.scalar_tensor_tensor(
                        out=ss[:, j, 1:2], in0=ss[:, j, 0:1],
                        scalar=ss[:, j, 0:1], in1=ss[:, j, 1:2],
                        op0=mybir.AluOpType.mult, op1=mybir.AluOpType.add)

                ps_g = ps.tile([G, 2], F32, tag="work", bufs=2)
                for j in range(NCHUNK):
                    nc.tensor.matmul(ps_g, lhsT=ind1[:, j, :], rhs=ss[:, j, :],
                                     start=(j == 0), stop=(j == NCHUNK - 1))
                # group stats -> mean_g, rstd_g
                gm = sp.tile([G, 2], F32, tag="gn_gm")       # [mean_g, rstd_g]
                tmp_g = sp.tile([G, 2], F32, tag="gn_tmp")
                nc.vector.tensor_scalar_mul(gm, ps_g, 1.0 / GS)           # [mean, E2]
                nc.vector.tensor_tensor(out=tmp_g[:, 0:1], in0=gm[:, 0:1],
                                        in1=gm[:, 0:1], op=mybir.AluOpType.mult)
                nc.vector.tensor_tensor(out=tmp_g[:, 1:2], in0=gm[:, 1:2],
                                        in1=tmp_g[:, 0:1], op=mybir.AluOpType.subtract)
                nc.scalar.activation(out=tmp_g[:, 1:2], in_=tmp_g[:, 1:2],
                                     func=mybir.ActivationFunctionType.Ln,
                                     bias=eps_t, scale=1.0)
                nc.scalar.activation(out=gm[:, 1:2], in_=tmp_g[:, 1:2],
                                     func=mybir.ActivationFunctionType.Exp,
                                     scale=-0.5)   # rstd_g = (var+eps)^-0.5

                ab = sp.tile([128, NCHUNK, 2], F32, tag="gn_ab")
                for j in range(NCHUNK):
                    ps_bc = ps.tile([128, 2], F32, tag="work", bufs=2, name=f"ps_bc{j}")
                    nc.tensor.matmul(ps_bc, lhsT=ind2[:, j, :], rhs=gm,
                                     start=True, stop=True)
                    # A = rstd_c * norm_w ; B = norm_b - mean_c * A
                    nc.vector.tensor_tensor(out=ab[:, j, 0:1], in0=ps_bc[:, 1:2],
                                            in1=nwb[:, j, 0:1], op=mybir.AluOpType.mult)
                    nc.vector.scalar_tensor_tensor(
                        out=stats[:, 0, 0:1], in0=ps_bc[:, 0:1],
                        scalar=ab[:, j, 0:1], in1=nwb[:, j, 1:2],
                        op0=mybir.AluOpType.mult, op1=mybir.AluOpType.subtract)
                    nc.vector.tensor_scalar_mul(ab[:, j, 1:2],
                                                stats[:, 0, 0:1], -1.0)
                    if j % 2 == 0:
                        # ACT is idle during the GN head: apply xn there
                        nc.scalar.activation(
                            out=xn[:, j, :], in_=x_sb[:, j, :],
                            func=mybir.ActivationFunctionType.Identity,
                            bias=ab[:, j, 1:2], scale=ab[:, j, 0:1])
                    else:
                        nc.vector.tensor_scalar(
                            out=xn[:, j, :], in0=x_sb[:, j, :],
                            scalar1=ab[:, j, 0:1], scalar2=ab[:, j, 1:2],
                            op0=mybir.AluOpType.mult,
                            op1=mybir.AluOpType.add)

                # ---------- stages B+C: software-pipelined qkv + attention ----
                # PE is in-order and co-saturated with ACT, so each head
                # iteration interleaves stall-free "filler" matmul steps
                # (qk(h+1), AV(h-1), vT chains) between the scores pairs,
                # which are the only PE ops that wait on ACT (psum reuse).
                # Chains never span an iteration (pool round-robin + in-order
                # PE would deadlock); fillers are emitted chain-by-chain.
                p_tiles = {}

                def qk_steps(h):
                    steps = []
                    for w in range(2):  # 0=q, 1=k
                        dst = q_sb if w == 0 else k_sb
                        for tt in range(2):
                            cell = {}

                            def step(jp, w=w, tt=tt, dst=dst, cell=cell, h=h):
                                if jp == 0:
                                    cell['ps'] = ps.tile(
                                        [128, 512], F32, tag="work", bufs=2,
                                        name=f"ps_qk{h}_{w}_{tt}")
                                nc.tensor.matmul(
                                    cell['ps'],
                                    lhsT=wqkv8[:, 2 * jp:2 * jp + 2,
                                               w * QCOLS + h * DP:
                                               w * QCOLS + (h + 1) * DP],
                                    rhs=xn[:, 2 * jp:2 * jp + 2,
                                           tt * 512:(tt + 1) * 512],
                                    start=(jp == 0), stop=(jp == 2),
                                    perf_mode=DR)
                                if jp == 2:
                                    nc.vector.tensor_copy(
                                        out=dst[:, h, tt * 512:(tt + 1) * 512],
                                        in_=cell['ps'])
                            steps += [lambda jp=jp, s=step: s(jp)
                                      for jp in range(3)]
                    return steps

                def vT_steps(nn, scs):  # nn: 4-head group; scs: sc chunks
                    steps = []
                    for sc in scs:
                        cell = {}

                        def step(jp, sc=sc, nn=nn, cell=cell):
                            if jp == 0:
                                cell['ps'] = ps.tile(
                                    [128, 4 * D], F32, tag="work", bufs=2,
                                    name=f"ps_v{sc}_{nn}")
                            nc.tensor.matmul(
                                cell['ps'],
                                lhsT=xn[:, 2 * jp:2 * jp + 2,
                                        sc * 128:(sc + 1) * 128],
                                rhs=wqkv8[:, 2 * jp:2 * jp + 2,
                                          VBASE + nn * 4 * D:
                                          VBASE + (nn + 1) * 4 * D],
                                start=(jp == 0), stop=(jp == 2),
                                perf_mode=DR)
                            if jp == 2:
                                nc.vector.tensor_copy(
                                    out=vT[:, nn * 4:(nn + 1) * 4, sc, 0:D],
                                    in_=cell['ps'].rearrange(
                                        "p (h d) -> p h d", h=4))
                        steps += [lambda jp=jp, s=step: s(jp) for jp in range(3)]
                    return steps

                def av_parts(h, tag="av"):
                    """AV chains + softmax-denominator trailer for head h.

                    Returns (chain, trailers): chain[tt][scp] and trailers[tt]
                    as zero-arg fns. chain steps honor DVE_PAIRS: offloaded
                    pairs read the bf16 probs tile with two plain matmuls
                    instead of one fp8 DoubleRow matmul.

                    The denominator uses reciprocal_approx_fast (fp32, ~18
                    good bits — InstReciprocal measured ~5x slower on HW) and
                    the trailer multiplies are deferred a full head so the
                    ~3us DRAM-bounce broadcast latency stays off the DVE
                    critical path."""
                    p_t, p_bf, dvp = p_tiles.pop(h)
                    rinv = sp.tile([1, T], F32, tag="rinv", name=f"rinv{h}")
                    rb = sp.tile([D, T], F32, tag="rb", name=f"rb{h}")
                    r_dr = dr.tile([1, T], F32, tag="rbounce", name=f"r_dr{h}")
                    cells = [{}, {}]
                    chain = [[], []]
                    for tt in range(2):
                        sl = slice(tt * 512, (tt + 1) * 512)
                        cell = cells[tt]

                        def step(scp, sl=sl, cell=cell, h=h, p_t=p_t,
                                 p_bf=p_bf, dvp=dvp, rinv=rinv, rb=rb,
                                 r_dr=r_dr, tt=tt, tag=tag):
                            if scp == 0:
                                cell['ps'] = ps.tile(
                                    [VW, 512], F32, tag=tag,
                                    name=f"ps_a{h}_{tt}", bufs=2)
                            ps_a = cell['ps']
                            if scp in dvp:
                                pi = dvp.index(scp)
                                for sub in range(2):
                                    nc.tensor.matmul(
                                        ps_a,
                                        lhsT=vT[:, h, 2 * scp + sub, :],
                                        rhs=p_bf[:, pi, sub, sl],
                                        start=(scp == 0 and sub == 0),
                                        stop=(scp == 3 and sub == 1))
                            else:
                                nc.tensor.matmul(
                                    ps_a,
                                    lhsT=vT[:, h, 2 * scp:2 * scp + 2, :],
                                    rhs=p_t[:, 2 * scp:2 * scp + 2, sl],
                                    start=(scp == 0), stop=(scp == 3),
                                    perf_mode=DR)
                            if scp == 3 and AV_VARIANT != "C":
                                # 1/d = exp(-ln d): both in ACT's exp table
                                # set, so no table reload amid the probs exp
                                # stream. (InstReciprocal on DVE measured
                                # ~2.7us per [1,512] — 43us/iter — and the
                                # custom-DVE approx ops don't compile here.)
                                lnd = sp.tile([1, T], F32, tag="lnd",
                                              name=f"lnd{h}")
                                nc.scalar.activation(
                                    out=lnd[0:1, sl], in_=ps_a[96:97, :],
                                    func=mybir.ActivationFunctionType.Ln,
                                    scale=1.0)
                                nc.scalar.activation(
                                    out=rinv[0:1, sl], in_=lnd[0:1, sl],
                                    func=mybir.ActivationFunctionType.Exp,
                                    scale=-1.0)
                                if h != NH - 1:
                                    # launch the DRAM-bounce broadcast now;
                                    # the consuming multiply runs a head
                                    # later, hiding the ~3us round trip
                                    nc.sync.dma_start(out=r_dr[:, sl],
                                                      in_=rinv[:, sl])
                                    nc.sync.dma_start(
                                        out=rb[:, sl],
                                        in_=r_dr[0:1, sl].to_broadcast(
                                            [D, 512]))
                        chain[tt] = [lambda scp=scp, s=step: s(scp)
                                     for scp in range(4)]

                    def trailer(tt, h=h, rinv=rinv, rb=rb, cells=cells):
                        sl = slice(tt * 512, (tt + 1) * 512)
                        if AV_VARIANT == "C":
                            nc.vector.tensor_copy(
                                out=a_sb[0:D, h, sl],
                                in_=cells[tt]['ps'][0:D, :])
                            return
                        if h == NH - 1:
                            # PE-broadcast + ACT evacuation (low latency).
                            # psum tag: "av"/"work" hold live AV cells here —
                            # allocating them would deadlock pool rotation.
                            ps_rb = ps.tile([D, 512], F32, tag="scores",
                                            bufs=2, name=f"ps_rb{h}_{tt}")
                            nc.tensor.matmul(ps_rb, lhsT=ones80,
                                             rhs=rinv[0:1, sl],
                                             start=True, stop=True)
                            nc.scalar.copy(out=rb[:, sl], in_=ps_rb)
                        nc.vector.tensor_tensor(
                            out=a_sb[0:D, h, sl], in0=cells[tt]['ps'][0:D, :],
                            in1=rb[:, sl], op=mybir.AluOpType.mult)

                    return chain, [lambda: trailer(0), lambda: trailer(1)]

                def run_iter(h, fillers, post=()):
                    """post: list of (sc_pos, [fns]) run right after sc_pos's
                    exp is emitted (sc_pos=NSC: after the loop). Positioned a
                    slot late by callers so the next scores pair is already
                    in the PE queue and ACT never bubbles."""
                    dvp = tuple(DVE_PAIRS.get(h, ()))
                    p_t = pp.tile([128, NSC, T], F8, tag="probs", name=f"p_t{h}")
                    p_bf = None
                    if dvp:
                        p_bf = pp.tile([128, len(dvp), 2, T], BF16,
                                       tag="pbf", name=f"p_bf{h}")
                    p_tiles[h] = (p_t, p_bf, dvp)
                    post_map = {}
                    for pos, fns in post:
                        post_map.setdefault(pos, []).extend(fns)
                    nf = len(fillers)
                    done = 0
                    for sc in range(NSC):
                        want = (sc * nf) // NSC
                        while done < want:
                            fillers[done]()
                            done += 1
                        ps_s = ps.tile([128, T], F32, tag="scores",
                                       name=f"ps_s{h}_{sc}", bufs=2)
                        for tt in range(2):
                            nc.tensor.matmul(
                                ps_s[:, tt * 512:(tt + 1) * 512],
                                lhsT=k_sb[:, h, sc * 128:(sc + 1) * 128],
                                rhs=q_sb[:, h, tt * 512:(tt + 1) * 512],
                                start=True, stop=True)
                        if sc // 2 in dvp:
                            pi = dvp.index(sc // 2)
                            nc.vector.tensor_scalar(
                                out=p_bf[:, pi, sc % 2, :].bitcast(
                                    mybir.dt.int16),
                                in0=ps_s,
                                scalar1=float(128.0 * LOG2E * EXPSCALE),
                                scalar2=float(16256.0 + 128.0 * LOG2E
                                              * EXPSHIFT - 5.0),
                                op0=mybir.AluOpType.mult,
                                op1=mybir.AluOpType.add)
                        else:
                            nc.scalar.activation(
                                out=p_t[:, sc, :], in_=ps_s,
                                func=mybir.ActivationFunctionType.Exp,
                                bias=neg2, scale=float(EXPSCALE))
                        for fn in post_map.pop(sc, ()):
                            fn()
                    while done < nf:
                        fillers[done]()
                        done += 1
                    for fn in post_map.pop(NSC, ()):
                        fn()

                if phase == 2:
                    # timing-attribution subset: all qkv/vT chains, no attn
                    for h in range(NH):
                        for s in qk_steps(h):
                            s()
                    for s in vT_steps(0, range(NSC)) + vT_steps(1, range(NSC)):
                        s()
                elif phase == 3:
                    for s in qk_steps(0):
                        s()
                    run_iter(0, qk_steps(1) + vT_steps(0, range(NSC)))
                    nn1 = vT_steps(1, range(NSC))
                    for h in range(1, NH):
                        fillers = []
                        if h + 1 < NH:
                            fillers += qk_steps(h + 1)
                        if 1 <= h <= 4:
                            fillers += nn1[(h - 1) * 6:h * 6]
                        run_iter(h, fillers)
                        p_tiles.pop(h - 1, None)
                elif phase >= 4:
                    for s in qk_steps(0):
                        s()
                    run_iter(0, qk_steps(1) + vT_steps(0, range(NSC)))
                    nn1 = vT_steps(1, range(NSC))  # 24 steps: spread h=1..4
                    trail = {}  # h -> deferred trailer fns
                    for h in range(1, NH):
                        # filler order matters: trailer mults (whose rb bounce
                        # is already down) first, then AV chains early so this
                        # head's rinv + bounce launch with maximal slack, then
                        # qk evacs (only needed by next head's scores)
                        fillers = []
                        if h - 2 in trail:
                            fillers += trail.pop(h - 2)
                        chain, trailers = av_parts(h - 1)
                        fillers += chain[0] + chain[1]
                        trail[h - 1] = trailers
                        if h + 1 < NH:
                            fillers += qk_steps(h + 1)
                        post = ()
                        if h == NH - 1:
                            # last head: AV(7) rides inside the exp window,
                            # positioned a slot after its probs pair; its
                            # psum uses the now-idle "work" tag so it never
                            # contends with AV(6) in the "av" tag
                            deferred = {}

                            def defer(idx, tt=None, h=h):
                                def go():
                                    if 'c' not in deferred:
                                        deferred['c'], deferred['t'] = \
                                            av_parts(h, tag="work")
                                    if tt is None:
                                        for t2 in range(2):
                                            deferred['c'][t2][idx]()
                                    else:
                                        deferred['t'][tt]()
                                return go

                            post = [(2, [defer(0)]), (4, [defer(1)]),
                                    (6, [defer(2)]),
                                    (NSC, [defer(3), defer(None, 0),
                                           defer(None, 1)])]
                        if 1 <= h <= 4:
                            fillers += nn1[(h - 1) * 6:h * 6]
                        run_iter(h, fillers, post=post)
                    for fn in trail.pop(NH - 2):  # trailers(6) at tail
                        fn()

                # ---------- stage D: proj + bias + residual ----------
                for j in range(NCHUNK if phase >= 5 else 0):
                    # bufs=5: with 2, out_t reuse chains each evacuation
                    # behind the previous chunk's DMA store (~13us tail)
                    out_t = sp.tile([128, T], F32, tag="out", name=f"out{j}",
                                    bufs=5)
                    for tt in range(2):
                        ptag = ("work", "scores", "av")[(2 * j + tt) % 3]
                        ps_p = ps.tile([128, 512], F32, tag=ptag,
                                       bufs=2,
                                       name=f"ps_p{j}_{tt}")
                        for hp in range(4):
                            nc.tensor.matmul(
                                ps_p,
                                lhsT=pwT8[:, 2 * hp:2 * hp + 2, j * 128:(j + 1) * 128],
                                rhs=a_sb[:, 2 * hp:2 * hp + 2, tt * 512:(tt + 1) * 512],
                                start=(hp == 0), stop=(hp == 3),
                                perf_mode=DR)
                        nc.vector.scalar_tensor_tensor(
                            out=out_t[:, tt * 512:(tt + 1) * 512], in0=ps_p,
                            scalar=1.0 / (WSCALE * WSCALE),
                            in1=x_sb[:, j, tt * 512:(tt + 1) * 512],
                            op0=mybir.AluOpType.mult, op1=mybir.AluOpType.add)
                    oeng = nc.scalar if j % 2 == 0 else nc.sync
                    oeng.dma_start(out=o_dv[:, j, :], in_=out_t)

    if split_mw:
        _split_multiwait(nc)
    return nc


_NC_CACHE = {}


def _get_program(repeat=1, loop_n=0):
    key = (repeat, loop_n)
    if key not in _NC_CACHE:
        _NC_CACHE[key] = _build_program(repeat, loop_n)
    return _NC_CACHE[key]


def _prep_shared(norm_w, norm_b, qkv_w, qkv_b, proj_w, proj_b):
    qkv_w = np.asarray(qkv_w, dtype=np.float32)
    proj_w = np.asarray(proj_w, dtype=np.float32)
    qkv_b = np.asarray(qkv_b, dtype=np.float32)
    proj_b = np.asarray(proj_b, dtype=np.float32)

    wq = qkv_w.reshape(3, NH, D, C)
    qb = qkv_b.reshape(3, NH, D)
    wqkv8 = np.zeros((CP, NCOLS), dtype=np.float32)
    for w in range(2):  # q, k: padded-to-128 head blocks
        for h in range(NH):
            base = w * QCOLS + h * DP
            wqkv8[0:C, base:base + D] = WSCALE * wq[w, h].T
            wqkv8[C, base:base + D] = WSCALE * qb[w, h]
    for h in range(NH):  # v: 80-wide head blocks
        base = VBASE + h * D
        wqkv8[0:C, base:base + D] = WSCALE * wq[2, h].T
        wqkv8[C, base:base + D] = WSCALE * qb[2, h]

    pwT8 = np.zeros((97, NH, C), dtype=np.float32)
    pwT8[0:D] = WSCALE * proj_w.reshape(C, NH, D).transpose(2, 1, 0)
    pwT8[96, 0, :] = WSCALE * proj_b

    cidx = np.arange(C) // GS
    ind1 = np.zeros((C, G), dtype=np.float32)
    ind1[np.arange(C), cidx] = 1.0
    ind2 = np.ascontiguousarray(ind1.T)
    return {
        "ind1": ind1,
        "ind2": ind2,
        "wqkv8": np.ascontiguousarray(wqkv8).astype(ml_dtypes.float8_e4m3),
        "pwT8": np.ascontiguousarray(pwT8).astype(ml_dtypes.float8_e4m3),
        "nw": np.ascontiguousarray(np.asarray(norm_w, dtype=np.float32)),
        "nb": np.ascontiguousarray(np.asarray(norm_b, dtype=np.float32)),
    }


def make_in_maps(x, norm_w, norm_b, qkv_w, qkv_b, proj_w, proj_b):
    x = np.asarray(x, dtype=np.float32)
    shared = _prep_shared(norm_w, norm_b, qkv_w, qkv_b, proj_w, proj_b)
    xs = x.reshape(B, C, T)
    return [dict(shared, x=np.ascontiguousarray(xs[i])) for i in range(B)]


def kernel(x, norm_w, norm_b, qkv_w, qkv_b, proj_w, proj_b):
    nc = _get_program()
    in_maps = make_in_maps(x, norm_w, norm_b, qkv_w, qkv_b, proj_w, proj_b)
    res = run_bass_kernel_spmd(nc, in_maps, core_ids=list(range(B)), trace=False)
    out = np.stack([res.results[i]["o"].reshape(C, HH, WW) for i in range(B)])
    return out.astype(np.float32)



# revision 35
# speedup vs baseline: 1.3004x; 1.0259x over previous
"""GAttentionBlock (GroupNorm + 8-head self-attention + proj + residual) on 8
Trainium2 NeuronCores, data-parallel over batch (B=8 -> 1 image per core).

v3 on top of the fp8(e4m3)+DoubleRow v2 baseline (183.8us -> ~133us HW):
  - Softmax denominator rework (the dominant v2 cost, ~63us/iter measured by
    ablation): DVE InstReciprocal (~2.7us per [1,512] on HW, 5x the model)
    is replaced by 1/d = exp(-ln d) on ACT — ln and exp share one ACT table
    set, so the probs-exp stream needs no table reloads. The [80,T]
    rinv broadcast still DRAM-bounces (SBUF APs cannot be partition-stride
    0), in bf16, but its consuming multiply is deferred a full head so the
    ~3us round trip is off every critical path (ablation: bounce now costs
    ~0 vs a constant-rb variant).
  - ~1/4 of the probs exps run on DVE as a piecewise-linear bf16 bit trick:
    bits = int16(128*log2e*(scale*x+shift) + 16250.7) viewed as bf16 equals
    exp(scale*x+shift) within +-3.5%; softmax renormalization cancels most
    of it (measured end-to-end err unchanged). t stays in [14.5k, 17.1k] for
    any plausible score so the int16 convert can never wrap. Offloaded pairs
    use two plain matmuls (bf16 probs rhs vs fp8 vT lhsT) in the AV chain.
    Mid-head pairs only: an end-of-head pair gates the next head's scores on
    the DVE queue drain (~1.8us ACT bubble per head).
  - AV(7) is interleaved into head 7's exp window via positioned post-steps
    ("work"-tag psum; one slot after its probs pair so ACT never bubbles),
    and the out stage got 5 out_t buffers + split store queues so the tail
    is DMA-drain-bound instead of buffer-recycle-bound.
  - x loads split across the SP/ACT DMA queues; filler order per head is
    [deferred trailer mults, AV chains, next-head qk chains] so rinv + its
    bounce launch with maximal slack.
Engine budget per iteration (timeline-sim, HW tracks it within ~15%):
ACT ~74us (48 exps + 32 ln/exp + GN/xn), DVE ~72us (evacs, 14 DVE-exps,
trailer mults, GN), PE ~59us, all overlapped to ~112us + For_i barrier.
"""
import copy

import numpy as np
import ml_dtypes

import concourse.bass as bass
import concourse.mybir as mybir
import concourse.tile as tile
from concourse.bass_utils import run_bass_kernel_spmd

F32 = mybir.dt.float32
BF16 = mybir.dt.bfloat16
F8 = mybir.dt.float8e4

B, C, HH, WW = 8, 640, 32, 32
T = HH * WW            # 1024
NH, D = 8, 80          # heads, head dim
DP = 128               # padded head dim (q/k)
G = 32                 # groupnorm groups
GS = C // G            # 20 channels per group
EPS = 1e-5
NCHUNK = C // 128      # 5 channel chunks of 128
CP = 768               # padded channels (bias row at 640)
NCHP = CP // 128       # 6
NSC = T // 128         # 8 sequence chunks of 128
VW = 112               # vT row width (80 v + pad + ones col at 96)
QCOLS = NH * DP        # 1024
VBASE = 2 * QCOLS      # 2048
NCOLS = 2 * QCOLS + NH * D  # 2688
WSCALE = 16.0
EXPSHIFT = -2.0
EXPSCALE = 1.0 / (WSCALE * WSCALE * np.sqrt(np.float64(D)))

_MAXW = 1
GP_XN = False  # gpsimd tensor_scalar measured ~15us on HW (10x the model) — keep Q7 out of the loop

# sc-pair indices (of 4 per head) whose exp runs on DVE as a piecewise-linear
# bf16 bit-trick instead of ACT exp: bits = int16(128*log2e*(scale*x+shift)
# + 16319.6) reinterpreted as bf16 ~= exp(scale*x+shift) within +-3.5%.
# t is always in [14500, 17100] for any plausible score, so the int16
# convert can neither underflow nor overflow: no clamp needed.
# pair 1 (mid-head): an end-of-head pair would gate the next head's first
# scores on the DVE queue draining, bubbling ACT ~1.8us per head
DVE_PAIRS: dict[int, tuple[int, ...]] = {
    h: ((1,) if h % 2 else (2,)) for h in range(1, NH)}
LOG2E = float(np.log2(np.e))

# AV-stage timing-attribution variants (hwav.py): A=DMA-bounce broadcast,
# C=no denominator, E=constant rb / no bounce (C/E timing-only, wrong numerics)
AV_VARIANT = "A"
RB_MERGE = False  # one [D,T] broadcast per head vs per-512-half


def _split_multiwait(nc):
    """This walrus build rejects >1 sync-wait command per instruction. Move
    extra waits onto same-engine NoOps inserted just before the instruction."""
    ctr = 0
    new_module = copy.replace(nc.m, functions=[])
    for function in nc.m.functions:
        new_function = copy.replace(function, blocks=[])
        new_function.set_allocations_from_list(function.allocations)
        for block in function.blocks:
            new_insts = []
            for inst in block.instructions:
                si = inst.sync_info
                ow = list(si.on_wait) if (si is not None and si.on_wait) else []
                if len(ow) > _MAXW:
                    head, tail = ow[:-_MAXW], ow[-_MAXW:]
                    for w in head:
                        ctr += 1
                        new_insts.append(mybir.InstNoOp(
                            name=f"mwsplit_{ctr}",
                            engine=inst.engine,
                            sync_info=mybir.SyncInfo(on_wait=[w], on_update=[]),
                            bass_nofuse=True,
                        ))
                    inst.sync_info = mybir.SyncInfo(
                        on_wait=tail,
                        on_update=list(si.on_update) if si.on_update else [],
                    )
                new_insts.append(inst)
            new_function.blocks.append(copy.replace(block, instructions=new_insts))
        new_module.functions.append(new_function)
    nc.m = new_module


def _build_program(repeat=1, loop_n=0, split_mw=True, phase=5, no_init=False):
    # no_init: sim-only — skip the one-time gpsimd memsets (they live outside
    # the For_i loop on HW, so the per-iteration body never pays them)
    nc = bass.Bass("TRN2", target_bir_lowering=False, num_devices=8)
    DR = mybir.MatmulPerfMode.DoubleRow

    x_d = nc.dram_tensor("x", [C, T], F32, kind="ExternalInput").ap()
    wqkv_d = nc.dram_tensor("wqkv8", [CP, NCOLS], F8, kind="ExternalInput").ap()
    pwT_d = nc.dram_tensor("pwT8", [97, NH, C], F8, kind="ExternalInput").ap()
    nw_d = nc.dram_tensor("nw", [C], F32, kind="ExternalInput").ap()
    nb_d = nc.dram_tensor("nb", [C], F32, kind="ExternalInput").ap()
    ind1_d = nc.dram_tensor("ind1", [C, G], F32, kind="ExternalInput").ap()
    ind2_d = nc.dram_tensor("ind2", [G, C], F32, kind="ExternalInput").ap()
    o_d = nc.dram_tensor("o", [C, T], F32, kind="ExternalOutput").ap()

    x_dv = x_d.rearrange("(o p) t -> p o t", p=128)       # [128, 5, 1024]
    o_dv = o_d.rearrange("(o p) t -> p o t", p=128)

    with tile.TileContext(nc) as tc:
        with tc.tile_pool(name="wpool", bufs=1) as wp, \
             tc.tile_pool(name="data", bufs=1) as dp, \
             tc.tile_pool(name="ptile", bufs=2) as pp, \
             tc.tile_pool(name="small", bufs=2) as sp, \
             tc.tile_pool(name="ps", bufs=2, space="PSUM") as ps, \
             tc.tile_pool(name="dram", bufs=2, space="DRAM") as dr:

            # ---------- weight / constant loads (ACT DMA queue: keeps the
            # SP queue free for the per-iteration x loads) ----------
            nwb = wp.tile([128, NCHUNK, 2], F32)
            ind1 = wp.tile([128, NCHUNK, G], F32)   # [channel -> group] one-hot
            ind2 = wp.tile([G, NCHUNK, 128], F32)   # [group -> channel] one-hot
            wqkv8 = wp.tile([128, NCHP, NCOLS], F8)
            pwT8 = wp.tile([97, NH, C], F8)
            eps_t = wp.tile([G, 1], F32)
            neg2 = wp.tile([128, 1], F32)
            ones80 = wp.tile([1, D], BF16)
            if not no_init:
                nc.scalar.dma_start(out=nwb[:, :, 0], in_=nw_d.rearrange("(o p) -> p o", p=128))
                nc.scalar.dma_start(out=nwb[:, :, 1], in_=nb_d.rearrange("(o p) -> p o", p=128))
                nc.scalar.dma_start(out=ind1, in_=ind1_d.rearrange("(o p) g -> p o g", p=128))
                nc.scalar.dma_start(out=ind2, in_=ind2_d.rearrange("g (o p) -> g o p", p=128))
                nc.scalar.dma_start(out=wqkv8, in_=wqkv_d.rearrange("(o p) n -> p o n", p=128))
                nc.scalar.dma_start(out=pwT8, in_=pwT_d)
                nc.vector.memset(eps_t, EPS)
                nc.vector.memset(neg2, EXPSHIFT)
                nc.vector.memset(ones80, 1.0)
            else:
                # sim-only: tiny touch-writes so Tile sees every one-time
                # tile allocated without the 18us of Pool memsets
                for _t in (nwb[0:1, 0:1, 0:1], ind1[0:1, 0:1, 0:1],
                           ind2[0:1, 0:1, 0:1], wqkv8[0:1, 0:1, 0:1],
                           pwT8[0:1, 0:1, 0:1], eps_t[0:1, 0:1],
                           neg2[0:1, 0:1], ones80[0:1, 0:1]):
                    nc.gpsimd.memset(_t, 0.0)

            # ---------- persistent data tiles + one-time inits ----------
            # x/xn double-buffered across repeat-bodies so body k+1's input
            # load + GN overlap body k's attention phase
            nxb = min(repeat, 2)
            xn_bufs, x_bufs = [], []
            for bi in range(nxb):
                xnb = dp.tile([128, NCHP, T], F8, name=f"xn{bi}")
                if not no_init:
                    nc.gpsimd.memset(xnb[:, NCHP - 1, :], 0.0)
                    nc.gpsimd.memset(xnb[0:1, NCHP - 1, :], 1.0)  # bias row 640
                else:
                    nc.gpsimd.memset(xnb[0:1, NCHP - 1, 0:1], 1.0)
                xn_bufs.append(xnb)
                x_bufs.append(dp.tile([128, NCHUNK, T], F32, name=f"x_sb{bi}"))
            vT = dp.tile([128, NH, NSC, VW], F8)
            a_sb = dp.tile([97, NH, T], F8)
            if not no_init:
                nc.gpsimd.memset(vT[:, :, :, D:VW], 0.0)
                nc.gpsimd.memset(vT[:, :, :, 96:97], 1.0)     # denominator ones col
                nc.gpsimd.memset(a_sb[64:97, :, :], 0.0)
                nc.gpsimd.memset(a_sb[96:97, :, :], 1.0)      # proj bias row
            else:
                nc.gpsimd.memset(vT[0:1, 0:1, 0:1, 0:1], 1.0)
                nc.gpsimd.memset(a_sb[0:1, 0:1, 0:1], 1.0)
            q_sb = dp.tile([128, NH, T], BF16)
            k_sb = dp.tile([128, NH, T], BF16)

            import contextlib
            loop_cm = tc.For_i(0, loop_n, 1) if loop_n else contextlib.nullcontext()
            with loop_cm:
              for _rep in range(repeat):
                xn = xn_bufs[_rep % nxb]
                x_sb = x_bufs[_rep % nxb]
                # ---------- stage A: load x + GroupNorm ----------
                # split across two DMA queues so the head isn't serialized
                # behind a single queue's 5-chunk stream
                for j in range(NCHUNK):
                    eng = nc.sync if j % 2 == 0 else nc.scalar
                    eng.dma_start(out=x_sb[:, j, :], in_=x_dv[:, j, :])

                stats = sp.tile([128, 2, 6], F32, tag="gn_stats")
                ss = sp.tile([128, NCHUNK, 2], F32, tag="gn_ss")
                for j in range(NCHUNK):
                    nc.vector.bn_stats(out=stats[:, 0, :], in_=x_sb[:, j, 0:512])
                    nc.vector.bn_stats(out=stats[:, 1, :], in_=x_sb[:, j, 512:1024])
                    nc.vector.bn_aggr(out=ss[:, j, :], in_=stats)
                    # ss[...,1] currently var; make it var + mean^2 = E[x^2]
                    nc.vector.scalar_tensor_tensor(
                        out=ss[:, j, 1:2], in0=ss[:, j, 0:1],
                        scalar=ss[:, j, 0:1], in1=ss[:, j, 1:2],
                        op0=mybir.AluOpType.mult, op1=mybir.AluOpType.add)

                ps_g = ps.tile([G, 2], F32, tag="work", bufs=2)
                for j in range(NCHUNK):
                    nc.tensor.matmul(ps_g, lhsT=ind1[:, j, :], rhs=ss[:, j, :],
                                     start=(j == 0), stop=(j == NCHUNK - 1))
                # group stats -> mean_g, rstd_g
                gm = sp.tile([G, 2], F32, tag="gn_gm")       # [mean_g, rstd_g]
                tmp_g = sp.tile([G, 2], F32, tag="gn_tmp")
                nc.vector.tensor_scalar_mul(gm, ps_g, 1.0 / GS)           # [mean, E2]
                nc.vector.tensor_tensor(out=tmp_g[:, 0:1], in0=gm[:, 0:1],
                                        in1=gm[:, 0:1], op=mybir.AluOpType.mult)
                nc.vector.tensor_tensor(out=tmp_g[:, 1:2], in0=gm[:, 1:2],
                                        in1=tmp_g[:, 0:1], op=mybir.AluOpType.subtract)
                nc.scalar.activation(out=tmp_g[:, 1:2], in_=tmp_g[:, 1:2],
                                     func=mybir.ActivationFunctionType.Ln,
                                     bias=eps_t, scale=1.0)
                nc.scalar.activation(out=gm[:, 1:2], in_=tmp_g[:, 1:2],
                                     func=mybir.ActivationFunctionType.Exp,
                                     scale=-0.5)   # rstd_g = (var+eps)^-0.5

                ab = sp.tile([128, NCHUNK, 2], F32, tag="gn_ab")
                for j in range(NCHUNK):
                    ps_bc = ps.tile([128, 2], F32, tag="work", bufs=2, name=f"ps_bc{j}")
                    nc.tensor.matmul(ps_bc, lhsT=ind2[:, j, :], rhs=gm,
                                     start=True, stop=True)
                    # A = rstd_c * norm_w ; B = norm_b - mean_c * A
                    nc.vector.tensor_tensor(out=ab[:, j, 0:1], in0=ps_bc[:, 1:2],
                                            in1=nwb[:, j, 0:1], op=mybir.AluOpType.mult)
                    nc.vector.scalar_tensor_tensor(
                        out=stats[:, 0, 0:1], in0=ps_bc[:, 0:1],
                        scalar=ab[:, j, 0:1], in1=nwb[:, j, 1:2],
                        op0=mybir.AluOpType.mult, op1=mybir.AluOpType.subtract)
                    nc.vector.tensor_scalar_mul(ab[:, j, 1:2],
                                                stats[:, 0, 0:1], -1.0)
                    if j % 2 == 0:
                        # ACT is idle during the GN head: apply xn there
                        nc.scalar.activation(
                            out=xn[:, j, :], in_=x_sb[:, j, :],
                            func=mybir.ActivationFunctionType.Identity,
                            bias=ab[:, j, 1:2], scale=ab[:, j, 0:1])
                    else:
                        nc.vector.tensor_scalar(
                            out=xn[:, j, :], in0=x_sb[:, j, :],
                            scalar1=ab[:, j, 0:1], scalar2=ab[:, j, 1:2],
                            op0=mybir.AluOpType.mult,
                            op1=mybir.AluOpType.add)

                # ---------- stages B+C: software-pipelined qkv + attention ----
                # PE is in-order and co-saturated with ACT, so each head
                # iteration interleaves stall-free "filler" matmul steps
                # (qk(h+1), AV(h-1), vT chains) between the scores pairs,
                # which are the only PE ops that wait on ACT (psum reuse).
                # Chains never span an iteration (pool round-robin + in-order
                # PE would deadlock); fillers are emitted chain-by-chain.
                p_tiles = {}

                def qk_steps(h):
                    steps = []
                    for w in range(2):  # 0=q, 1=k
                        dst = q_sb if w == 0 else k_sb
                        for tt in range(2):
                            cell = {}

                            def step(jp, w=w, tt=tt, dst=dst, cell=cell, h=h):
                                if jp == 0:
                                    cell['ps'] = ps.tile(
                                        [128, 512], F32, tag="work", bufs=2,
                                        name=f"ps_qk{h}_{w}_{tt}")
                                nc.tensor.matmul(
                                    cell['ps'],
                                    lhsT=wqkv8[:, 2 * jp:2 * jp + 2,
                                               w * QCOLS + h * DP:
                                               w * QCOLS + (h + 1) * DP],
                                    rhs=xn[:, 2 * jp:2 * jp + 2,
                                           tt * 512:(tt + 1) * 512],
                                    start=(jp == 0), stop=(jp == 2),
                                    perf_mode=DR)
                                if jp == 2:
                                    nc.vector.tensor_copy(
                                        out=dst[:, h, tt * 512:(tt + 1) * 512],
                                        in_=cell['ps'])
                            steps += [lambda jp=jp, s=step: s(jp)
                                      for jp in range(3)]
                    return steps

                def vT_steps(nn, scs):  # nn: 4-head group; scs: sc chunks
                    steps = []
                    for sc in scs:
                        cell = {}

                        def step(jp, sc=sc, nn=nn, cell=cell):
                            if jp == 0:
                                cell['ps'] = ps.tile(
                                    [128, 4 * D], F32, tag="work", bufs=2,
                                    name=f"ps_v{sc}_{nn}")
                            nc.tensor.matmul(
                                cell['ps'],
                                lhsT=xn[:, 2 * jp:2 * jp + 2,
                                        sc * 128:(sc + 1) * 128],
                                rhs=wqkv8[:, 2 * jp:2 * jp + 2,
                                          VBASE + nn * 4 * D:
                                          VBASE + (nn + 1) * 4 * D],
                                start=(jp == 0), stop=(jp == 2),
                                perf_mode=DR)
                            if jp == 2:
                                nc.vector.tensor_copy(
                                    out=vT[:, nn * 4:(nn + 1) * 4, sc, 0:D],
                                    in_=cell['ps'].rearrange(
                                        "p (h d) -> p h d", h=4))
                        steps += [lambda jp=jp, s=step: s(jp) for jp in range(3)]
                    return steps

                def av_parts(h, tag="av"):
                    """AV chains + softmax-denominator trailer for head h.

                    Returns (chain, trailers): chain[tt][scp] and trailers[tt]
                    as zero-arg fns. chain steps honor DVE_PAIRS: offloaded
                    pairs read the bf16 probs tile with two plain matmuls
                    instead of one fp8 DoubleRow matmul.

                    The denominator uses reciprocal_approx_fast (fp32, ~18
                    good bits — InstReciprocal measured ~5x slower on HW) and
                    the trailer multiplies are deferred a full head so the
                    ~3us DRAM-bounce broadcast latency stays off the DVE
                    critical path."""
                    p_t, p_bf, dvp = p_tiles.pop(h)
                    rinv = sp.tile([1, T], BF16, tag="rinv", name=f"rinv{h}")
                    rb = sp.tile([D, T], BF16, tag="rb", name=f"rb{h}")
                    r_dr = dr.tile([1, T], BF16, tag="rbounce", name=f"r_dr{h}")
                    if AV_VARIANT == "E" and h != NH - 1:
                        nc.gpsimd.memset(rb[0:1, 0:1], 1.0)  # timing-only
                    cells = [{}, {}]
                    chain = [[], []]
                    for tt in range(2):
                        sl = slice(tt * 512, (tt + 1) * 512)
                        cell = cells[tt]

                        def step(scp, sl=sl, cell=cell, h=h, p_t=p_t,
                                 p_bf=p_bf, dvp=dvp, rinv=rinv, rb=rb,
                                 r_dr=r_dr, tt=tt, tag=tag):
                            if scp == 0:
                                cell['ps'] = ps.tile(
                                    [VW, 512], F32, tag=tag,
                                    name=f"ps_a{h}_{tt}", bufs=2)
                            ps_a = cell['ps']
                            if scp in dvp:
                                pi = dvp.index(scp)
                                for sub in range(2):
                                    nc.tensor.matmul(
                                        ps_a,
                                        lhsT=vT[:, h, 2 * scp + sub, :],
                                        rhs=p_bf[:, pi, sub, sl],
                                        start=(scp == 0 and sub == 0),
                                        stop=(scp == 3 and sub == 1))
                            else:
                                nc.tensor.matmul(
                                    ps_a,
                                    lhsT=vT[:, h, 2 * scp:2 * scp + 2, :],
                                    rhs=p_t[:, 2 * scp:2 * scp + 2, sl],
                                    start=(scp == 0), stop=(scp == 3),
                                    perf_mode=DR)
                            if scp == 3 and AV_VARIANT != "C":
                                # 1/d = exp(-ln d): both in ACT's exp table
                                # set, so no table reload amid the probs exp
                                # stream. (InstReciprocal on DVE measured
                                # ~2.7us per [1,512] — 43us/iter — and the
                                # custom-DVE approx ops don't compile here.)
                                lnd = sp.tile([1, T], F32, tag="lnd",
                                              name=f"lnd{h}")
                                nc.scalar.activation(
                                    out=lnd[0:1, sl], in_=ps_a[96:97, :],
                                    func=mybir.ActivationFunctionType.Ln,
                                    scale=1.0)
                                nc.scalar.activation(
                                    out=rinv[0:1, sl], in_=lnd[0:1, sl],
                                    func=mybir.ActivationFunctionType.Exp,
                                    scale=-1.0)
                                if (h != NH - 1 and AV_VARIANT != "E"
                                        and (tt == 1 or not RB_MERGE)):
                                    # DRAM-bounce broadcast (SBUF APs cannot
                                    # have stride-0 partitions), bf16. The
                                    # consuming multiply runs a head later,
                                    # hiding the latency.
                                    bsl = slice(None) if RB_MERGE else sl
                                    wid = T if RB_MERGE else 512
                                    nc.sync.dma_start(out=r_dr[:, bsl],
                                                      in_=rinv[:, bsl])
                                    nc.sync.dma_start(
                                        out=rb[:, bsl],
                                        in_=r_dr[0:1, bsl].to_broadcast(
                                            [D, wid]))
                        chain[tt] = [lambda scp=scp, s=step: s(scp)
                                     for scp in range(4)]

                    def trailer(tt, h=h, rinv=rinv, rb=rb, cells=cells):
                        sl = slice(tt * 512, (tt + 1) * 512)
                        if AV_VARIANT == "C":
                            nc.vector.tensor_copy(
                                out=a_sb[0:D, h, sl],
                                in_=cells[tt]['ps'][0:D, :])
                            return
                        if h == NH - 1:
                            # PE-broadcast + ACT evacuation (low latency).
                            # psum tag: "av"/"work" hold live AV cells here —
                            # allocating them would deadlock pool rotation.
                            ps_rb = ps.tile([D, 512], F32, tag="scores",
                                            bufs=2, name=f"ps_rb{h}_{tt}")
                            nc.tensor.matmul(ps_rb, lhsT=ones80,
                                             rhs=rinv[0:1, sl],
                                             start=True, stop=True)
                            nc.scalar.copy(out=rb[:, sl], in_=ps_rb)
                        nc.vector.tensor_tensor(
                            out=a_sb[0:D, h, sl], in0=cells[tt]['ps'][0:D, :],
                            in1=rb[:, sl], op=mybir.AluOpType.mult)

                    return chain, [lambda: trailer(0), lambda: trailer(1)]

                def run_iter(h, fillers, post=(), ramp=False):
                    """post: list of (sc_pos, [fns]) run right after sc_pos's
                    exp is emitted (sc_pos=NSC: after the loop). Positioned a
                    slot late by callers so the next scores pair is already
                    in the PE queue and ACT never bubbles. ramp=True defers
                    fillers quadratically (head 0: ACT must catch up to PE
                    through the 2-deep scores-psum rotation first)."""
                    dvp = tuple(DVE_PAIRS.get(h, ()))
                    p_t = pp.tile([128, NSC, T], F8, tag="probs", name=f"p_t{h}")
                    p_bf = None
                    if dvp:
                        p_bf = pp.tile([128, len(dvp), 2, T], BF16,
                                       tag="pbf", name=f"p_bf{h}")
                    p_tiles[h] = (p_t, p_bf, dvp)
                    post_map = {}
                    for pos, fns in post:
                        post_map.setdefault(pos, []).extend(fns)
                    nf = len(fillers)
                    done = 0
                    for sc in range(NSC):
                        want = ((sc * sc * nf) // (NSC * NSC) if ramp
                                else (sc * nf) // NSC)
                        while done < want:
                            fillers[done]()
                            done += 1
                        ps_s = ps.tile([128, T], F32, tag="scores",
                                       name=f"ps_s{h}_{sc}", bufs=2)
                        for tt in range(2):
                            nc.tensor.matmul(
                                ps_s[:, tt * 512:(tt + 1) * 512],
                                lhsT=k_sb[:, h, sc * 128:(sc + 1) * 128],
                                rhs=q_sb[:, h, tt * 512:(tt + 1) * 512],
                                start=True, stop=True)
                        if sc // 2 in dvp:
                            pi = dvp.index(sc // 2)
                            nc.vector.tensor_scalar(
                                out=p_bf[:, pi, sc % 2, :].bitcast(
                                    mybir.dt.int16),
                                in0=ps_s,
                                scalar1=float(128.0 * LOG2E * EXPSCALE),
                                scalar2=float(16256.0 + 128.0 * LOG2E
                                              * EXPSHIFT - 5.0),
                                op0=mybir.AluOpType.mult,
                                op1=mybir.AluOpType.add)
                        else:
                            nc.scalar.activation(
                                out=p_t[:, sc, :], in_=ps_s,
                                func=mybir.ActivationFunctionType.Exp,
                                bias=neg2, scale=float(EXPSCALE))
                        for fn in post_map.pop(sc, ()):
                            fn()
                    while done < nf:
                        fillers[done]()
                        done += 1
                    for fn in post_map.pop(NSC, ()):
                        fn()

                if phase == 2:
                    # timing-attribution subset: all qkv/vT chains, no attn
                    for h in range(NH):
                        for s in qk_steps(h):
                            s()
                    for s in vT_steps(0, range(NSC)) + vT_steps(1, range(NSC)):
                        s()
                elif phase == 3:
                    for s in qk_steps(0):
                        s()
                    run_iter(0, vT_steps(0, range(NSC)) + qk_steps(1))
                    nn1 = vT_steps(1, range(NSC))
                    for h in range(1, NH):
                        fillers = []
                        if h + 1 < NH:
                            fillers += qk_steps(h + 1)
                        if 1 <= h <= 4:
                            fillers += nn1[(h - 1) * 6:h * 6]
                        run_iter(h, fillers)
                        p_tiles.pop(h - 1, None)
                elif phase >= 4:
                    for s in qk_steps(0):
                        s()
                    run_iter(0, vT_steps(0, range(NSC)) + qk_steps(1))
                    nn1 = vT_steps(1, range(NSC))  # 24 steps: spread h=1..4
                    trail = {}  # h -> deferred trailer fns
                    for h in range(1, NH):
                        # filler order matters: trailer mults (whose rb bounce
                        # is already down) first, then AV chains early so this
                        # head's rinv + bounce launch with maximal slack, then
                        # qk evacs (only needed by next head's scores)
                        fillers = []
                        if h - 2 in trail:
                            fillers += trail.pop(h - 2)
                        chain, trailers = av_parts(h - 1)
                        fillers += chain[0] + chain[1]
                        trail[h - 1] = trailers
                        if h + 1 < NH:
                            fillers += qk_steps(h + 1)
                        post = ()
                        if h == NH - 1:
                            # last head: AV(7) rides inside the exp window,
                            # positioned a slot after its probs pair; its
                            # psum uses the now-idle "work" tag so it never
                            # contends with AV(6) in the "av" tag
                            deferred = {}

                            def defer(idx, tt=None, h=h):
                                def go():
                                    if 'c' not in deferred:
                                        deferred['c'], deferred['t'] = \
                                            av_parts(h, tag="work")
                                    if tt is None:
                                        for t2 in range(2):
                                            deferred['c'][t2][idx]()
                                    else:
                                        deferred['t'][tt]()
                                return go

                            post = [(2, [defer(0)]), (4, [defer(1)]),
                                    (6, [defer(2)]),
                                    (NSC, [defer(3), defer(None, 0),
                                           defer(None, 1)])]
                        if 1 <= h <= 4:
                            fillers += nn1[(h - 1) * 6:h * 6]
                        run_iter(h, fillers, post=post)
                    for fn in trail.pop(NH - 2):  # trailers(6) at tail
                        fn()

                # ---------- stage D: proj + bias + residual ----------
                for j in range(NCHUNK if phase >= 5 else 0):
                    # bufs=5: with 2, out_t reuse chains each evacuation
                    # behind the previous chunk's DMA store (~13us tail)
                    out_t = sp.tile([128, T], F32, tag="out", name=f"out{j}",
                                    bufs=5)
                    for tt in range(2):
                        ptag = ("work", "scores", "av")[(2 * j + tt) % 3]
                        ps_p = ps.tile([128, 512], F32, tag=ptag,
                                       bufs=2,
                                       name=f"ps_p{j}_{tt}")
                        for hp in range(4):
                            nc.tensor.matmul(
                                ps_p,
                                lhsT=pwT8[:, 2 * hp:2 * hp + 2, j * 128:(j + 1) * 128],
                                rhs=a_sb[:, 2 * hp:2 * hp + 2, tt * 512:(tt + 1) * 512],
                                start=(hp == 0), stop=(hp == 3),
                                perf_mode=DR)
                        nc.vector.scalar_tensor_tensor(
                            out=out_t[:, tt * 512:(tt + 1) * 512], in0=ps_p,
                            scalar=1.0 / (WSCALE * WSCALE),
                            in1=x_sb[:, j, tt * 512:(tt + 1) * 512],
                            op0=mybir.AluOpType.mult, op1=mybir.AluOpType.add)
                    oeng = nc.scalar if j % 2 == 0 else nc.sync
                    oeng.dma_start(out=o_dv[:, j, :], in_=out_t)

    if split_mw:
        _split_multiwait(nc)
    return nc


_NC_CACHE = {}


def _get_program(repeat=1, loop_n=0):
    key = (repeat, loop_n)
    if key not in _NC_CACHE:
        _NC_CACHE[key] = _build_program(repeat, loop_n)
    return _NC_CACHE[key]


def _prep_shared(norm_w, norm_b, qkv_w, qkv_b, proj_w, proj_b):
    qkv_w = np.asarray(qkv_w, dtype=np.float32)
    proj_w = np.asarray(proj_w, dtype=np.float32)
    qkv_b = np.asarray(qkv_b, dtype=np.float32)
    proj_b = np.asarray(proj_b, dtype=np.float32)

    wq = qkv_w.reshape(3, NH, D, C)
    qb = qkv_b.reshape(3, NH, D)
    wqkv8 = np.zeros((CP, NCOLS), dtype=np.float32)
    for w in range(2):  # q, k: padded-to-128 head blocks
        for h in range(NH):
            base = w * QCOLS + h * DP
            wqkv8[0:C, base:base + D] = WSCALE * wq[w, h].T
            wqkv8[C, base:base + D] = WSCALE * qb[w, h]
    for h in range(NH):  # v: 80-wide head blocks
        base = VBASE + h * D
        wqkv8[0:C, base:base + D] = WSCALE * wq[2, h].T
        wqkv8[C, base:base + D] = WSCALE * qb[2, h]

    pwT8 = np.zeros((97, NH, C), dtype=np.float32)
    pwT8[0:D] = WSCALE * proj_w.reshape(C, NH, D).transpose(2, 1, 0)
    pwT8[96, 0, :] = WSCALE * proj_b

    cidx = np.arange(C) // GS
    ind1 = np.zeros((C, G), dtype=np.float32)
    ind1[np.arange(C), cidx] = 1.0
    ind2 = np.ascontiguousarray(ind1.T)
    return {
        "ind1": ind1,
        "ind2": ind2,
        "wqkv8": np.ascontiguousarray(wqkv8).astype(ml_dtypes.float8_e4m3),
        "pwT8": np.ascontiguousarray(pwT8).astype(ml_dtypes.float8_e4m3),
        "nw": np.ascontiguousarray(np.asarray(norm_w, dtype=np.float32)),
        "nb": np.ascontiguousarray(np.asarray(norm_b, dtype=np.float32)),
    }


def make_in_maps(x, norm_w, norm_b, qkv_w, qkv_b, proj_w, proj_b):
    x = np.asarray(x, dtype=np.float32)
    shared = _prep_shared(norm_w, norm_b, qkv_w, qkv_b, proj_w, proj_b)
    xs = x.reshape(B, C, T)
    return [dict(shared, x=np.ascontiguousarray(xs[i])) for i in range(B)]


def kernel(x, norm_w, norm_b, qkv_w, qkv_b, proj_w, proj_b):
    nc = _get_program()
    in_maps = make_in_maps(x, norm_w, norm_b, qkv_w, qkv_b, proj_w, proj_b)
    res = run_bass_kernel_spmd(nc, in_maps, core_ids=list(range(B)), trace=False)
    out = np.stack([res.results[i]["o"].reshape(C, HH, WW) for i in range(B)])
    return out.astype(np.float32)

